# revision 49
# baseline (speedup 1.0000x reference)
"""AttentionPooling Trainium2 kernel (8 NeuronCores, Bass/Tile).

Sharding: (batch, head-group) — core c handles batch b=c//2 and heads
4*(c%2)..4*(c%2)+3. Each core computes, for its 4 heads, Q^T/K^T (head-dim
major) projections and V (token major), then a one-pass pooled attention:

  For each query stripe of 128 rows:  S = Q_stripe K^T  (PE, bf16)
  E = exp(S/(1024*sqrt(d))) (ScalarE, accum_out -> Z), r = 1/Z (VectorE)
  wacc[j, c] += onehot_j(r)^T E[:, j*512+c]  (PE, per-head [4,512] PSUM
  accumulator; the [4,128] block m transposes to w columns of k-tile 4j+m)

  attended_h = sum_t wT[k-tile t]^T V[t]  (PE, one-hot [4,128] accumulator)
  pooled = concat_h(attended) @ Wo_slice^T / N   (folded mean-pool)

Numerics: Q/K projections run fp8(e4m3) x fp8 with weights pre-scaled by
32 on the host (the 1/1024 folds into the exp scale) — fp8 matmuls run at
bf16 speed, but halve the critical-path DMA bytes. V stays bf16 (fp8 Wv
error does NOT average out through w@V); host-verified max rel err 5.6e-3
vs the fp32 reference. The K bias is dropped: it only adds a per-query
constant to the scores, which softmax cancels. V/output biases fold on the
host: pooled += Wo@bv + bo.

Schedule: a dummy exp preloads the ACT table and a few junk matmuls warm
the PE clock-gate while the critical DMAs land (fp8 x + h0 weights; the
bf16 x / Wv / Wo loads are dependency-deferred behind the prologue so they
don't steal DMA bandwidth). K(h0)+Q(h0,c0) project as a prologue; the
remaining Q/K chunks and V tiles interleave between attention stripes via
a cost/deadline-paced background queue. Per-head w finalizes (transpose to
wT) while the next head's stripes run; attended matmuls are slotted at
stripes 49/53/57; only head 3's finalize+attend and the 8 pooled matmuls
trail the last stripe.

PSUM (8 banks): S stripes 2x[128,1024] (4) + projection chunks 2x[128,512]
(2, also lent to small finalize tiles) + w/attended accumulators 2x (2).
"""

import math
import sys

import numpy as np

for _p in ("/opt/trn_rl_repo",):
    if _p not in sys.path:
        sys.path.append(_p)

import ml_dtypes

B, N, HID = 4, 2048, 1024
HEADS, HD = 8, 128
NH = 4          # heads per core
HGW = NH * HD   # head-group width (512)
NCORES = 8
P = 128
QT_TILES = N // P    # 16 query stripes per head
TOK_TILES = N // P   # 16 token tiles
NCHUNK = 4           # 512-token projection chunks

BF16 = ml_dtypes.bfloat16
F8 = ml_dtypes.float8_e4m3  # TRN fp8e4: max 240

_cache = {}


def _build_nc():
    import concourse.bacc as bacc
    import concourse.tile as tile
    from concourse import mybir
    from concourse.bass import ds, ts
    from concourse.masks import make_identity
    from concourse.tile import add_dep_helper

    BF = mybir.dt.bfloat16
    F32 = mybir.dt.float32
    FP8 = mybir.dt.float8e4
    AF = mybir.ActivationFunctionType

    nc = bacc.Bacc(trn_type="TRN2")

    # fp8 x, token-chunk major: xq8[pi, c, po, n'] = x[b, c*512+n', po*128+pi]
    xq8_d = nc.dram_tensor("xq8", (P, NCHUNK, 8, 512), FP8, kind="ExternalInput").ap()
    # fp8 Q/K weights (x32): wq8[pi, h, po, d] = 32*Wq[hg*512+h*128+d, po*128+pi]
    wq8_d = nc.dram_tensor("wq8", (P, NH, 8, P), FP8, kind="ExternalInput").ap()
    wk8_d = nc.dram_tensor("wk8", (P, NH, 8, P), FP8, kind="ExternalInput").ap()
    # bf16 x, hid-tile major: xbf[pi, po, n] = x[b, n, po*128+pi]
    xbf_d = nc.dram_tensor("xbf", (P, 8, N), BF, kind="ExternalInput").ap()
    # bf16 V weights: wvb[pi, po, o] = Wv[hg*512+o, po*128+pi]
    wvb_d = nc.dram_tensor("wvb", (P, 8, HGW), BF, kind="ExternalInput").ap()
    # bf16 O weights: wob[pi, h, o] = Wo[o, hg*512+h*128+pi]
    wob_d = nc.dram_tensor("wob", (P, NH, HID), BF, kind="ExternalInput").ap()
    bq_d = nc.dram_tensor("bq32", (P, NH), F32, kind="ExternalInput").ap()
    out_d = nc.dram_tensor("out_pooled", (1, HID), F32, kind="ExternalOutput").ap()

    inv_exp = float(1.0 / (1024.0 * math.sqrt(HD)))
    inv_pool = float(1.0 / N)

    with tile.TileContext(nc) as tc:
        with (
            tc.tile_pool(name="persist", bufs=1) as persist,
            tc.tile_pool(name="sp", bufs=2, space="PSUM") as sp,
            tc.tile_pool(name="pp", bufs=2, space="PSUM") as pp,
            tc.tile_pool(name="wp", bufs=2, space="PSUM") as wp,
            tc.tile_pool(name="ep", bufs=3) as ep,
            tc.tile_pool(name="zp", bufs=4) as zp,
        ):
            # ---- input DMAs, emitted first so the queues start at t0 ----
            # Per-queue FIFO tiering (no dep-gating — dep-gated DMAs degrade
            # to descriptor-at-a-time trickle): tier 1 = K(h0,c0) operands
            # across all 16 queues, tier 2 = remaining token chunks, tier 3
            # = everything the background projections need later.
            xq8_sb = [
                persist.tile([P, 8, 512], FP8, name=f"xq8_{i}")
                for i in range(NCHUNK)
            ]
            wq8_sb = [
                persist.tile([P, 8, P], FP8, name=f"wq8_{i}") for i in range(NH)
            ]
            wk8_sb = [
                persist.tile([P, 8, P], FP8, name=f"wk8_{i}") for i in range(NH)
            ]
            xbf_sb = persist.tile([P, 8, N], BF)
            wvb_sb = persist.tile([P, 8, HGW], BF)
            wob_sb = persist.tile([P, NH, HID], BF)
            bq_sb = persist.tile([P, NH], F32)

            def dma_split(dst, src_, nsplit):
                step = P // nsplit
                for i in range(nsplit):
                    nc.sync.dma_start(
                        out=dst[i * step : (i + 1) * step],
                        in_=src_[i * step : (i + 1) * step],
                    )

            # Each dma_start costs ~0.6us of serial issue on the Sync queue
            # (DIRECT2D), so the count is minimized and ordered so K(h0,c0)'s
            # operands issue first; transfers overlap later issues.
            nc.sync.dma_start(out=wk8_sb[0], in_=wk8_d[:, 0])
            dma_split(xq8_sb[0], xq8_d[:, 0], 2)
            nc.sync.dma_start(out=wq8_sb[0], in_=wq8_d[:, 0])
            for c in range(1, NCHUNK):
                nc.sync.dma_start(out=xq8_sb[c], in_=xq8_d[:, c])
            nc.sync.dma_start(out=bq_sb, in_=bq_d)
            for h in range(1, NH):
                nc.sync.dma_start(out=wk8_sb[h], in_=wk8_d[:, h])
                nc.sync.dma_start(out=wq8_sb[h], in_=wq8_d[:, h])
            for half in range(2):
                nc.sync.dma_start(
                    out=xbf_sb[:, 4 * half : 4 * half + 4, :],
                    in_=xbf_d[:, 4 * half : 4 * half + 4, :],
                )
            nc.sync.dma_start(out=wvb_sb, in_=wvb_d)
            nc.sync.dma_start(out=wob_sb, in_=wob_d)

            # ---- small constants (DVE) --------------------------------
            # mask16 columns {0,5,10,15} are 1: slicing [:, 4j:4j+4] gives
            # the one-hot column j used to route r into wacc row j.
            mask16 = persist.tile([P, 4 * NH], BF)
            nc.vector.memset(mask16, 0.0)
            for j in range(4):
                nc.vector.memset(mask16[:, 5 * j : 5 * j + 1], 1.0)
            zs128 = persist.tile([P, P], BF)
            nc.vector.memset(zs128, 0.0)
            ident4 = persist.tile([4, 4], F32)
            make_identity(nc, ident4)
            # 4x4 identity replicated at each 32-partition row group, so the
            # block transposes of wacc (stationary at base partition 32s) use
            # a moving operand at the same base partition.
            ident4x = persist.tile([P, 4], F32)
            nc.vector.memset(ident4x, 0.0)
            for s in range(4):
                nc.sync.dma_start(out=ident4x[32 * s : 32 * s + 4, :], in_=ident4)

            QT_sb = persist.tile([P, NH, N], BF)
            KT_sb = persist.tile([P, NH, N], BF)
            V_sb = persist.tile([P, TOK_TILES, HGW], BF)
            wacc_sb = persist.tile([P, 512], F32)
            # wT[pi, h, j, s, :]: [128,4] stationary for k-tile t=4j+s of
            # head h — one-hot at column j by construction (the transpose of
            # the block-diagonal wacc region), so head h's attended matmuls
            # accumulate partials into row j of a [4,128] PSUM tile.
            wT_sb = persist.tile([P, NH, 4, 4, 4], BF)
            att4_sb = persist.tile([4, P], F32)
            # attT2[:, h, oc, :]: [128,2] stationary with attended_h at
            # column oc (other column zero) so the two pooled-projection
            # matmuls of head h land in rows 0/1 of one [2,512] accumulator.
            attT2_sb = persist.tile([P, NH, 2, 2], BF)
            nc.vector.memset(attT2_sb, 0.0)
            pooled2_sb = persist.tile([2, 512], F32)

            # ---- ACT table preload + PE warmup (run under the DMAs) ---
            zdum = zp.tile([P, 16], BF, tag="zd", name="zdum")
            nc.scalar.activation(out=zdum, in_=mask16, func=AF.Exp)
            for _ in range(12):
                warm_ps = pp.tile([16, 512], F32, tag="proj", name="warm_ps")
                nc.tensor.matmul(
                    warm_ps, lhsT=mask16, rhs=KT_sb[:, 0, 0:512],
                    start=True, stop=True, skip_group_check=True,
                )

            # ---- projection emitters ----------------------------------
            def qk_chunk(proj_i, h, c, step=False):
                """512-token fp8 DoubleRow Q^T/K^T projection for head h:
                vitile v contracts hid pair-blocks (2v, 2v+1)."""
                wsb, dst = ((wq8_sb, QT_sb), (wk8_sb, KT_sb))[proj_i]
                ps = pp.tile([P, 512], F32, tag="proj", name="ps_qk")
                for v in range(4):
                    nc.tensor.matmul(
                        ps,
                        lhsT=wsb[h][:, 2 * v : 2 * v + 2, :],
                        rhs=xq8_sb[c][:, 2 * v : 2 * v + 2, :],
                        start=(v == 0),
                        stop=(v == 3),
                        perf_mode=mybir.MatmulPerfMode.DoubleRow,
                    )
                    if step and v == 1:
                        yield
                if proj_i == 0:
                    # Q bias (32*bq) folded into the psum->bf16 evacuation
                    ev = nc.vector.tensor_tensor(
                        dst[:, h, ts(c, 512)],
                        ps,
                        bq_sb[:, h : h + 1].to_broadcast((P, 512)),
                        mybir.AluOpType.add,
                    )
                else:
                    ev = nc.vector.tensor_copy(dst[:, h, ts(c, 512)], ps)
                if step:
                    yield ev

            def v_chunk(t, step=False):
                """128-token bf16 V projection tile (all 4 heads)."""
                ps = pp.tile([P, HGW], F32, tag="proj", name="ps_v")
                for i in range(8):
                    nc.tensor.matmul(
                        ps,
                        lhsT=xbf_sb[:, i, ts(t, P)],
                        rhs=wvb_sb[:, i, :],
                        start=(i == 0),
                        stop=(i == 7),
                    )
                    if step and i in (2, 5):
                        yield
                nc.vector.tensor_copy(V_sb[:, t, :], ps)
                if step:
                    yield

            # ---- prologue: K(h0) + Q(h0,c0) ---------------------------
            for c in range(NCHUNK):
                for _ in qk_chunk(1, 0, c):
                    pass
            for _ in qk_chunk(0, 0, 0):
                pass

            # ---- background queue: (generator, est_ns, deadline, nb) --
            bg = []
            for c in range(1, NCHUNK):
                bg.append((qk_chunk(0, 0, c, True), 1100.0, 4 * c - 2, 0))
            for h in range(1, NH):
                for c in range(NCHUNK):
                    bg.append((qk_chunk(1, h, c, True), 1100.0, 16 * h - 4 + c, 0))
                for c in range(NCHUNK):
                    bg.append(
                        (qk_chunk(0, h, c, True), 1100.0, 16 * h + 4 * c - 2, 0)
                    )
            for t in range(TOK_TILES):
                bg.append((v_chunk(t, True), 3000.0, 33 + t, 14 + t))
            bg_total = sum(u[1] for u in bg)
            bg_state = {"i": 0, "spent": 0.0}
            BG_SPREAD = 52  # finish all background work by stripe 52 of 64

            def bg_step():
                gen, cost, _, _ = bg[bg_state["i"]]
                try:
                    next(gen)
                    bg_state["spent"] += cost / 3.0
                except StopIteration:
                    bg_state["spent"] = sum(u[1] for u in bg[: bg_state["i"] + 1])
                    bg_state["i"] += 1

            def bg_advance(si):
                while bg_state["i"] < len(bg) and bg[bg_state["i"]][2] <= si + 1:
                    bg_step()
                target = (si + 1) * bg_total / BG_SPREAD
                while (
                    bg_state["i"] < len(bg)
                    and bg_state["spent"] < target
                    and bg[bg_state["i"]][3] <= si
                ):
                    bg_step()

            # ---- per-head finalize + attended (aux-paced) -------------
            wacc_tiles = {}
            pooled_tile = [None]

            def finalize(h):
                """wacc (PSUM, block-diag) -> wT_sb[:, h] one-hot k-tiles.
                The scale-copy (first step) releases the wacc pool slot; the
                16 transposes spread over the following stripes."""
                wps = wacc_tiles.pop(h)
                nc.vector.tensor_scalar_mul(wacc_sb, wps, inv_pool)
                yield
                for s in range(4):
                    for j in range(4):
                        tp = pp.tile([P, 4], F32, tag="proj", name="tp_ps")
                        nc.tensor.transpose(
                            tp,
                            wacc_sb[32 * s : 32 * s + 4, ts(j, P)],
                            ident4x[32 * s : 32 * s + 4, :],
                            tile_position=(32 * s, 0),
                        )
                        nc.vector.tensor_copy(wT_sb[:, h, j, s, :], tp)
                    yield

            def attend(h):
                """attended_h = sum_t wT[k-tile t]^T V[t, head h], then its
                two pooled-projection matmuls into the shared accumulator."""
                aps = pp.tile([4, P], F32, tag="proj", name="att4_ps")
                for t in range(TOK_TILES):
                    nc.tensor.matmul(
                        aps,
                        lhsT=wT_sb[:, h, t // 4, t % 4, :],
                        rhs=V_sb[:, t, ts(h, HD)],
                        start=(t == 0),
                        stop=(t == TOK_TILES - 1),
                    )
                    if t in (3, 7, 11):
                        yield
                nc.vector.tensor_copy(att4_sb, aps)
                atp = pp.tile([P, 4], F32, tag="proj", name="attT_ps")
                nc.tensor.transpose(atp, att4_sb, ident4)
                ar = zp.tile([P, 1], F32, tag="ar", name="attr")
                nc.vector.reduce_sum(ar, atp, axis=mybir.AxisListType.X)
                for oc in range(2):
                    nc.vector.tensor_copy(attT2_sb[:, h, oc, oc : oc + 1], ar)
                if pooled_tile[0] is None:
                    pooled_tile[0] = wp.tile([2, 512], F32, tag="w", name="pooled")
                for oc in range(2):
                    nc.tensor.matmul(
                        pooled_tile[0],
                        lhsT=attT2_sb[:, h, oc, :],
                        rhs=wob_sb[:, h, ts(oc, 512)],
                        start=(h == 0 and oc == 0),
                        stop=(h == NH - 1 and oc == 1),
                        skip_group_check=True,
                    )
                yield

            aux = []

            def aux_step(n=1):
                for _ in range(n):
                    while aux:
                        try:
                            next(aux[0])
                            break
                        except StopIteration:
                            aux.pop(0)

            # ---- pooled attention stripe loop -------------------------
            def emit_S(h, qi):
                tiles = []
                for kk in range(2):
                    s_ps = sp.tile([P, 1024], F32, tag="s", name="s_ps")
                    for kc in range(2):
                        nc.tensor.matmul(
                            s_ps[:, ts(kc, 512)],
                            lhsT=QT_sb[:, h, ts(qi, P)],
                            rhs=KT_sb[:, h, ds(kk * 1024 + kc * 512, 512)],
                            start=True,
                            stop=True,
                        )
                    tiles.append(s_ps)
                return tiles

            def emit_w(pend):
                # 16 [4,128] matmuls, 4-way col-group concurrent: region
                # (j, s) at partitions [32s, 32s+4), free [128j, 128j+128)
                # holds w[j*512+s*128+c] at row j (one-hot lhsT), i.e. the
                # [4,128] block (s, j) transposes to k-tile 4j+s.
                e_t, rb16, h, first, last = pend
                if first:
                    wacc_tiles[h] = wp.tile([P, 512], F32, tag="w", name="wacc")
                    # single full-bank zero-matmul opens the accumulation:
                    # start=True clearing is coarser than a [4,128] region,
                    # so per-region start bits would wipe sibling regions.
                    nc.tensor.matmul(
                        wacc_tiles[h],
                        lhsT=zs128,
                        rhs=e_t[:, 0:512],
                        start=True,
                        stop=False,
                        skip_group_check=True,
                    )
                wps = wacc_tiles[h]
                for j in range(4):
                    for s in range(4):
                        nc.tensor.matmul(
                            wps[32 * s : 32 * s + 4, ts(j, P)],
                            lhsT=rb16[:, 4 * j : 4 * j + 4],
                            rhs=e_t[:, ds(512 * j + 128 * s, P)],
                            start=False,
                            stop=last,
                            tile_position=(0, 32 * s),
                            skip_group_check=True,
                        )

            pend_s = emit_S(0, 0)
            pend_w = None
            for gi in range(NH * QT_TILES):
                e_t = ep.tile([P, N], BF, tag="e", name="e_t")
                zs = []
                for kk, s_ps in enumerate(pend_s):
                    z_t = zp.tile([P, 1], F32, tag=f"z{kk}", name="z_t")
                    nc.scalar.activation(
                        out=e_t[:, ts(kk, 1024)],
                        in_=s_ps,
                        func=AF.Exp,
                        scale=inv_exp,
                        accum_out=z_t,
                    )
                    zs.append(z_t)
                if gi + 1 < NH * QT_TILES:
                    pend_s = emit_S((gi + 1) // QT_TILES, (gi + 1) % QT_TILES)
                r_t = zp.tile([P, 1], F32, tag="r", name="r_t")
                nc.vector.tensor_add(r_t, zs[0], zs[1])
                nc.vector.reciprocal(r_t, r_t)
                rb16 = zp.tile([P, 4 * NH], BF, tag="rb", name="rb16")
                nc.vector.tensor_tensor(
                    rb16,
                    mask16,
                    r_t.to_broadcast((P, 4 * NH)),
                    mybir.AluOpType.mult,
                )
                bg_advance(gi)
                if pend_w is not None:
                    emit_w(pend_w)
                    if pend_w[4]:  # closed head pend_w[2]'s accumulator
                        aux.append(finalize(pend_w[2]))
                pend_w = (
                    e_t, rb16, gi // QT_TILES,
                    gi % QT_TILES == 0, gi % QT_TILES == QT_TILES - 1,
                )
                if gi == 49:
                    aux.append(attend(0))
                elif gi == 52:
                    aux.append(attend(1))
                elif gi == 55:
                    aux.append(attend(2))
                aux_step(n=2)

            emit_w(pend_w)
            aux.append(finalize(3))
            aux.append(attend(3))
            aux_step(n=100)
            nc.vector.tensor_copy(pooled2_sb, pooled_tile[0])
            nc.sync.dma_start(
                out=out_d.rearrange("a (b c) -> (a b) c", b=2),
                in_=pooled2_sb,
            )

    nc.finalize()
    return nc


def _get_nc():
    if "nc" not in _cache:
        _cache["nc"] = _build_nc()
    return _cache["nc"]


def _f8(a):
    return np.clip(a, -240.0, 240.0).astype(F8)


def _host_prep(inputs):
    """Build the 8 per-core input maps (shard + transpose + quantize)."""
    x = np.asarray(inputs["chunk_embeddings"], np.float32)
    wq = np.asarray(inputs["Wq"], np.float32)
    wk = np.asarray(inputs["Wk"], np.float32)
    wv = np.asarray(inputs["Wv"], np.float32)
    wo = np.asarray(inputs["Wo"], np.float32)
    bq = np.asarray(inputs["bq"], np.float32)
    in_maps = []
    for c in range(NCORES):
        b, hg = c // 2, c % 2
        sl = slice(hg * HGW, (hg + 1) * HGW)
        xT = x[b].T  # (1024, 2048): [po*128+pi, n]
        # xq8[pi, c, po, n'] = x[b, c*512+n', po*128+pi]
        xq8 = _f8(
            np.ascontiguousarray(
                xT.reshape(8, P, NCHUNK, 512).transpose(1, 2, 0, 3)
            )
        )
        # w?8[pi, h, po, d] = 32*W[hg*512+h*128+d, po*128+pi]
        def w8(W):
            m = (32.0 * W[sl, :]).T.reshape(8, P, NH, P).transpose(1, 2, 0, 3)
            return _f8(np.ascontiguousarray(m))
        # xbf[pi, po, n]
        xbf = np.ascontiguousarray(xT.reshape(8, P, N).transpose(1, 0, 2)).astype(
            BF16
        )
        # wvb[pi, po, o] = Wv[hg*512+o, po*128+pi]
        wvb = np.ascontiguousarray(
            wv[sl, :].T.reshape(8, P, HGW).transpose(1, 0, 2)
        ).astype(BF16)
        # wob[pi, h, o] = Wo[o, hg*512+h*128+pi]
        wob = np.ascontiguousarray(
            wo[:, sl].T.reshape(NH, P, HID).transpose(1, 0, 2)
        ).astype(BF16)
        bq32 = np.ascontiguousarray((32.0 * bq[sl]).reshape(NH, P).T)
        in_maps.append(
            {
                "xq8": xq8,
                "wq8": w8(wq),
                "wk8": w8(wk),
                "xbf": xbf,
                "wvb": wvb,
                "wob": wob,
                "bq32": bq32,
            }
        )
    return in_maps


def _unshard(results, inputs):
    bo = np.asarray(inputs["bo"], np.float32)
    bv = np.asarray(inputs["bv"], np.float32)
    Wo = np.asarray(inputs["Wo"], np.float32)
    bv_wo = Wo @ bv  # exact fold of the V bias through the output projection
    out = np.zeros((B, HID), np.float32)
    for b in range(B):
        out[b] = (
            results[2 * b]["out_pooled"][0]
            + results[2 * b + 1]["out_pooled"][0]
            + bv_wo
            + bo
        )
    return out


def _reference_numpy(inputs):
    """Fallback for non-trivial attention masks (never hit for the spec'd
    all-ones mask): straight numpy port of the reference."""
    x = np.asarray(inputs["chunk_embeddings"], np.float32)
    mask = np.asarray(inputs["attention_mask"])
    b, n, hid = x.shape

    def proj(W, bias):
        y = x @ np.asarray(W, np.float32).T + np.asarray(bias, np.float32)
        return y.reshape(b, n, HEADS, HD).transpose(0, 2, 1, 3)

    Q = proj(inputs["Wq"], inputs["bq"])
    K = proj(inputs["Wk"], inputs["bk"])
    V = proj(inputs["Wv"], inputs["bv"])
    s = np.einsum("bhqd,bhkd->bhqk", Q, K) / np.float32(np.sqrt(HD))
    s = np.where(mask[:, None, None, :] == 0, np.float32(-1e9), s)
    s = s - s.max(axis=-1, keepdims=True)
    e = np.exp(s)
    a = e / e.sum(axis=-1, keepdims=True)
    att = np.einsum("bhqk,bhkd->bhqd", a, V)
    att = att.transpose(0, 2, 1, 3).reshape(b, n, hid)
    out = att @ np.asarray(inputs["Wo"], np.float32).T + np.asarray(
        inputs["bo"], np.float32
    )
    m = mask[:, :, None].astype(np.float32)
    return (out * m).sum(axis=1) / m.sum(axis=1)


def _run(inputs, trace=False):
    from concourse.bass_utils import run_bass_kernel_spmd

    nc = _get_nc()
    in_maps = _host_prep(inputs)
    res = run_bass_kernel_spmd(
        nc, in_maps, core_ids=list(range(NCORES)), trace=trace
    )
    _cache["last_result"] = res
    return _unshard(res.results, inputs)


def kernel(**inputs):
    mask = np.asarray(inputs["attention_mask"])
    if not np.all(mask == 1):
        return _reference_numpy(inputs)
    return _run(inputs, trace=False)


def kernel_traced(**inputs):
    """Like kernel() but with NTFF profiling; returns (out, exec_time_ns)."""
    out = _run(inputs, trace=True)
    return out, _cache["last_result"].exec_time_ns


# revision 50
# speedup vs baseline: 1.0083x; 1.0083x over previous
"""AttentionPooling Trainium2 kernel (8 NeuronCores, Bass/Tile).

Sharding: (batch, head-group) — core c handles batch b=c//2 and heads
4*(c%2)..4*(c%2)+3. Each core computes, for its 4 heads, Q^T/K^T (head-dim
major) projections and V (token major), then a one-pass pooled attention:

  For each query stripe of 128 rows:  S = Q_stripe K^T  (PE, bf16)
  E = exp(S/(1024*sqrt(d))) (ScalarE, accum_out -> Z), r = 1/Z (VectorE)
  wacc[j, c] += onehot_j(r)^T E[:, j*512+c]  (PE, per-head [4,512] PSUM
  accumulator; the [4,128] block m transposes to w columns of k-tile 4j+m)

  attended_h = sum_t wT[k-tile t]^T V[t]  (PE, one-hot [4,128] accumulator)
  pooled = concat_h(attended) @ Wo_slice^T / N   (folded mean-pool)

Numerics: Q/K projections run fp8(e4m3) x fp8 with weights pre-scaled by
32 on the host (the 1/1024 folds into the exp scale) — fp8 matmuls run at
bf16 speed, but halve the critical-path DMA bytes. V stays bf16 (fp8 Wv
error does NOT average out through w@V); host-verified max rel err 5.6e-3
vs the fp32 reference. The K bias is dropped: it only adds a per-query
constant to the scores, which softmax cancels. V/output biases fold on the
host: pooled += Wo@bv + bo.

Schedule: a dummy exp preloads the ACT table and a few junk matmuls warm
the PE clock-gate while the critical DMAs land (fp8 x + h0 weights; the
bf16 x / Wv / Wo loads are dependency-deferred behind the prologue so they
don't steal DMA bandwidth). K(h0)+Q(h0,c0) project as a prologue; the
remaining Q/K chunks and V tiles interleave between attention stripes via
a cost/deadline-paced background queue. Per-head w finalizes (transpose to
wT) while the next head's stripes run; attended matmuls are slotted at
stripes 49/53/57; only head 3's finalize+attend and the 8 pooled matmuls
trail the last stripe.

PSUM (8 banks): S stripes 2x[128,1024] (4) + projection chunks 2x[128,512]
(2, also lent to small finalize tiles) + w/attended accumulators 2x (2).
"""

import math
import sys

import numpy as np

for _p in ("/opt/trn_rl_repo",):
    if _p not in sys.path:
        sys.path.append(_p)

import ml_dtypes

B, N, HID = 4, 2048, 1024
HEADS, HD = 8, 128
NH = 4          # heads per core
HGW = NH * HD   # head-group width (512)
NCORES = 8
P = 128
QT_TILES = N // P    # 16 query stripes per head
TOK_TILES = N // P   # 16 token tiles
NCHUNK = 4           # 512-token projection chunks

BF16 = ml_dtypes.bfloat16
F8 = ml_dtypes.float8_e4m3  # TRN fp8e4: max 240

_cache = {}


def _build_nc():
    import concourse.bacc as bacc
    import concourse.tile as tile
    from concourse import mybir
    from concourse.bass import ds, ts
    from concourse.masks import make_identity
    from concourse.tile import add_dep_helper

    BF = mybir.dt.bfloat16
    F32 = mybir.dt.float32
    FP8 = mybir.dt.float8e4
    AF = mybir.ActivationFunctionType

    nc = bacc.Bacc(trn_type="TRN2")

    # fp8 x, token-chunk major: xq8[pi, c, po, n'] = x[b, c*512+n', po*128+pi]
    xq8_d = nc.dram_tensor("xq8", (P, NCHUNK, 8, 512), FP8, kind="ExternalInput").ap()
    # fp8 Q/K weights (x32): wq8[pi, h, po, d] = 32*Wq[hg*512+h*128+d, po*128+pi]
    wq8_d = nc.dram_tensor("wq8", (P, NH, 8, P), FP8, kind="ExternalInput").ap()
    wk8_d = nc.dram_tensor("wk8", (P, NH, 8, P), FP8, kind="ExternalInput").ap()
    # bf16 x, hid-tile major: xbf[pi, po, n] = x[b, n, po*128+pi]
    xbf_d = nc.dram_tensor("xbf", (P, 8, N), BF, kind="ExternalInput").ap()
    # bf16 V weights: wvb[pi, po, o] = Wv[hg*512+o, po*128+pi]
    wvb_d = nc.dram_tensor("wvb", (P, 8, HGW), BF, kind="ExternalInput").ap()
    # bf16 O weights: wob[pi, h, o] = Wo[o, hg*512+h*128+pi]
    wob_d = nc.dram_tensor("wob", (P, NH, HID), BF, kind="ExternalInput").ap()
    bq_d = nc.dram_tensor("bq32", (P, NH), F32, kind="ExternalInput").ap()
    out_d = nc.dram_tensor("out_pooled", (1, HID), F32, kind="ExternalOutput").ap()

    inv_exp = float(1.0 / (1024.0 * math.sqrt(HD)))
    inv_pool = float(1.0 / N)

    with tile.TileContext(nc) as tc:
        with (
            tc.tile_pool(name="persist", bufs=1) as persist,
            tc.tile_pool(name="sp", bufs=2, space="PSUM") as sp,
            tc.tile_pool(name="pp", bufs=2, space="PSUM") as pp,
            tc.tile_pool(name="wp", bufs=2, space="PSUM") as wp,
            tc.tile_pool(name="ep", bufs=3) as ep,
            tc.tile_pool(name="zp", bufs=4) as zp,
        ):
            # ---- input DMAs, emitted first so the queues start at t0 ----
            # Per-queue FIFO tiering (no dep-gating — dep-gated DMAs degrade
            # to descriptor-at-a-time trickle): tier 1 = K(h0,c0) operands
            # across all 16 queues, tier 2 = remaining token chunks, tier 3
            # = everything the background projections need later.
            xq8_sb = [
                persist.tile([P, 8, 512], FP8, name=f"xq8_{i}")
                for i in range(NCHUNK)
            ]
            wq8_sb = [
                persist.tile([P, 8, P], FP8, name=f"wq8_{i}") for i in range(NH)
            ]
            wk8_sb = [
                persist.tile([P, 8, P], FP8, name=f"wk8_{i}") for i in range(NH)
            ]
            xbf_sb = persist.tile([P, 8, N], BF)
            wvb_sb = persist.tile([P, 8, HGW], BF)
            wob_sb = persist.tile([P, NH, HID], BF)
            bq_sb = persist.tile([P, NH], F32)

            def dma_split(dst, src_, nsplit):
                step = P // nsplit
                for i in range(nsplit):
                    nc.sync.dma_start(
                        out=dst[i * step : (i + 1) * step],
                        in_=src_[i * step : (i + 1) * step],
                    )

            # Each dma_start costs ~0.6us of serial issue on the Sync queue
            # (DIRECT2D), so the count is minimized and ordered so K(h0,c0)'s
            # operands issue first; transfers overlap later issues.
            nc.sync.dma_start(out=wk8_sb[0], in_=wk8_d[:, 0])
            dma_split(xq8_sb[0], xq8_d[:, 0], 2)
            nc.sync.dma_start(out=wq8_sb[0], in_=wq8_d[:, 0])
            for c in range(1, NCHUNK):
                nc.sync.dma_start(out=xq8_sb[c], in_=xq8_d[:, c])
            nc.sync.dma_start(out=bq_sb, in_=bq_d)
            for h in range(1, NH):
                nc.sync.dma_start(out=wk8_sb[h], in_=wk8_d[:, h])
                nc.sync.dma_start(out=wq8_sb[h], in_=wq8_d[:, h])
            for half in range(2):
                nc.sync.dma_start(
                    out=xbf_sb[:, 4 * half : 4 * half + 4, :],
                    in_=xbf_d[:, 4 * half : 4 * half + 4, :],
                )
            nc.sync.dma_start(out=wvb_sb, in_=wvb_d)
            nc.sync.dma_start(out=wob_sb, in_=wob_d)

            # ---- small constants (DVE) --------------------------------
            # mask16 columns {0,5,10,15} are 1: slicing [:, 4j:4j+4] gives
            # the one-hot column j used to route r into wacc row j.
            mask16 = persist.tile([P, 4 * NH], BF)
            nc.vector.memset(mask16, 0.0)
            for j in range(4):
                nc.vector.memset(mask16[:, 5 * j : 5 * j + 1], 1.0)
            zs128 = persist.tile([P, P], BF)
            nc.vector.memset(zs128, 0.0)
            ident4 = persist.tile([4, 4], F32)
            make_identity(nc, ident4)
            # 4x4 identity replicated at each 32-partition row group, so the
            # block transposes of wacc (stationary at base partition 32s) use
            # a moving operand at the same base partition.
            ident4x = persist.tile([P, 4], F32)
            nc.vector.memset(ident4x, 0.0)
            for s in range(4):
                nc.sync.dma_start(out=ident4x[32 * s : 32 * s + 4, :], in_=ident4)

            QT_sb = persist.tile([P, NH, N], BF)
            KT_sb = persist.tile([P, NH, N], BF)
            V_sb = persist.tile([P, TOK_TILES, HGW], BF)
            wacc_sb = persist.tile([P, 512], F32)
            # wT[pi, h, j, s, :]: [128,4] stationary for k-tile t=4j+s of
            # head h — one-hot at column j by construction (the transpose of
            # the block-diagonal wacc region), so head h's attended matmuls
            # accumulate partials into row j of a [4,128] PSUM tile.
            wT_sb = persist.tile([P, NH, 4, 4, 4], BF)
            att4_sb = persist.tile([4, P], F32)
            # attT2[:, h, oc, :]: [128,2] stationary with attended_h at
            # column oc (other column zero) so the two pooled-projection
            # matmuls of head h land in rows 0/1 of one [2,512] accumulator.
            attT2_sb = persist.tile([P, NH, 2, 2], BF)
            nc.vector.memset(attT2_sb, 0.0)
            pooled2_sb = persist.tile([2, 512], F32)

            # ---- ACT table preload + PE warmup (run under the DMAs) ---
            zdum = zp.tile([P, 16], BF, tag="zd", name="zdum")
            nc.scalar.activation(out=zdum, in_=mask16, func=AF.Exp)
            for _ in range(12):
                warm_ps = pp.tile([16, 512], F32, tag="proj", name="warm_ps")
                nc.tensor.matmul(
                    warm_ps, lhsT=mask16, rhs=KT_sb[:, 0, 0:512],
                    start=True, stop=True, skip_group_check=True,
                )

            # ---- projection emitters ----------------------------------
            def qk_chunk(proj_i, h, c, step=False):
                """512-token fp8 DoubleRow Q^T/K^T projection for head h:
                vitile v contracts hid pair-blocks (2v, 2v+1)."""
                wsb, dst = ((wq8_sb, QT_sb), (wk8_sb, KT_sb))[proj_i]
                ps = pp.tile([P, 512], F32, tag="proj", name="ps_qk")
                for v in range(4):
                    nc.tensor.matmul(
                        ps,
                        lhsT=wsb[h][:, 2 * v : 2 * v + 2, :],
                        rhs=xq8_sb[c][:, 2 * v : 2 * v + 2, :],
                        start=(v == 0),
                        stop=(v == 3),
                        perf_mode=mybir.MatmulPerfMode.DoubleRow,
                    )
                    if step and v == 1:
                        yield
                if proj_i == 0:
                    # Q bias (32*bq) folded into the psum->bf16 evacuation
                    ev = nc.vector.tensor_tensor(
                        dst[:, h, ts(c, 512)],
                        ps,
                        bq_sb[:, h : h + 1].to_broadcast((P, 512)),
                        mybir.AluOpType.add,
                    )
                else:
                    ev = nc.vector.tensor_copy(dst[:, h, ts(c, 512)], ps)
                if step:
                    yield ev

            def v_chunk(t, step=False):
                """128-token bf16 V projection tile (all 4 heads)."""
                ps = pp.tile([P, HGW], F32, tag="proj", name="ps_v")
                for i in range(8):
                    nc.tensor.matmul(
                        ps,
                        lhsT=xbf_sb[:, i, ts(t, P)],
                        rhs=wvb_sb[:, i, :],
                        start=(i == 0),
                        stop=(i == 7),
                    )
                    if step and i in (2, 5):
                        yield
                nc.vector.tensor_copy(V_sb[:, t, :], ps)
                if step:
                    yield

            # ---- prologue: K(h0) + Q(h0,c0) ---------------------------
            for c in range(NCHUNK):
                for _ in qk_chunk(1, 0, c):
                    pass
            for _ in qk_chunk(0, 0, 0):
                pass

            # ---- background queue: (generator, est_ns, deadline, nb) --
            bg = []
            for c in range(1, NCHUNK):
                bg.append((qk_chunk(0, 0, c, True), 1100.0, 4 * c - 2, 0))
            for h in range(1, NH):
                for c in range(NCHUNK):
                    bg.append((qk_chunk(1, h, c, True), 1100.0, 16 * h - 4 + c, 0))
                for c in range(NCHUNK):
                    bg.append(
                        (qk_chunk(0, h, c, True), 1100.0, 16 * h + 4 * c - 2, 0)
                    )
            for t in range(TOK_TILES):
                bg.append((v_chunk(t, True), 3000.0, 33 + t, 14 + t))
            bg_total = sum(u[1] for u in bg)
            bg_state = {"i": 0, "spent": 0.0}
            BG_SPREAD = 52  # finish all background work by stripe 52 of 64

            def bg_step():
                gen, cost, _, _ = bg[bg_state["i"]]
                try:
                    next(gen)
                    bg_state["spent"] += cost / 3.0
                except StopIteration:
                    bg_state["spent"] = sum(u[1] for u in bg[: bg_state["i"] + 1])
                    bg_state["i"] += 1

            def bg_advance(si):
                while bg_state["i"] < len(bg) and bg[bg_state["i"]][2] <= si + 1:
                    bg_step()
                target = (si + 1) * bg_total / BG_SPREAD
                while (
                    bg_state["i"] < len(bg)
                    and bg_state["spent"] < target
                    and bg[bg_state["i"]][3] <= si
                ):
                    bg_step()

            # ---- per-head finalize + attended (aux-paced) -------------
            wacc_tiles = {}
            pooled_tile = [None]

            def finalize(h):
                """wacc (PSUM, block-diag) -> wT_sb[:, h] one-hot k-tiles.
                The scale-copy (first step) releases the wacc pool slot; the
                16 transposes spread over the following stripes."""
                wps = wacc_tiles.pop(h)
                nc.vector.tensor_scalar_mul(wacc_sb, wps, inv_pool)
                yield
                for s in range(4):
                    for j in range(4):
                        tp = pp.tile([P, 4], F32, tag="proj", name="tp_ps")
                        nc.tensor.transpose(
                            tp,
                            wacc_sb[32 * s : 32 * s + 4, ts(j, P)],
                            ident4x[32 * s : 32 * s + 4, :],
                            tile_position=(32 * s, 0),
                        )
                        nc.vector.tensor_copy(wT_sb[:, h, j, s, :], tp)
                    yield

            def attend(h):
                """attended_h = sum_t wT[k-tile t]^T V[t, head h], then its
                two pooled-projection matmuls into the shared accumulator."""
                aps = pp.tile([4, P], F32, tag="proj", name="att4_ps")
                for t in range(TOK_TILES):
                    nc.tensor.matmul(
                        aps,
                        lhsT=wT_sb[:, h, t // 4, t % 4, :],
                        rhs=V_sb[:, t, ts(h, HD)],
                        start=(t == 0),
                        stop=(t == TOK_TILES - 1),
                    )
                    if t in (3, 7, 11):
                        yield
                nc.vector.tensor_copy(att4_sb, aps)
                atp = pp.tile([P, 4], F32, tag="proj", name="attT_ps")
                nc.tensor.transpose(atp, att4_sb, ident4)
                ar = zp.tile([P, 1], F32, tag="ar", name="attr")
                nc.vector.reduce_sum(ar, atp, axis=mybir.AxisListType.X)
                for oc in range(2):
                    nc.vector.tensor_copy(attT2_sb[:, h, oc, oc : oc + 1], ar)
                if pooled_tile[0] is None:
                    pooled_tile[0] = wp.tile([2, 512], F32, tag="w", name="pooled")
                for oc in range(2):
                    nc.tensor.matmul(
                        pooled_tile[0],
                        lhsT=attT2_sb[:, h, oc, :],
                        rhs=wob_sb[:, h, ts(oc, 512)],
                        start=(h == 0 and oc == 0),
                        stop=(h == NH - 1 and oc == 1),
                        skip_group_check=True,
                    )
                yield

            aux = []

            def aux_step(n=1):
                for _ in range(n):
                    while aux:
                        try:
                            next(aux[0])
                            break
                        except StopIteration:
                            aux.pop(0)

            # ---- pooled attention stripe loop -------------------------
            def emit_S(h, qi):
                tiles = []
                for kk in range(2):
                    s_ps = sp.tile([P, 1024], F32, tag="s", name="s_ps")
                    for kc in range(2):
                        nc.tensor.matmul(
                            s_ps[:, ts(kc, 512)],
                            lhsT=QT_sb[:, h, ts(qi, P)],
                            rhs=KT_sb[:, h, ds(kk * 1024 + kc * 512, 512)],
                            start=True,
                            stop=True,
                        )
                    tiles.append(s_ps)
                return tiles

            def emit_w(pend):
                # 16 [4,128] matmuls, 4-way col-group concurrent: region
                # (j, s) at partitions [32s, 32s+4), free [128j, 128j+128)
                # holds w[j*512+s*128+c] at row j (one-hot lhsT), i.e. the
                # [4,128] block (s, j) transposes to k-tile 4j+s.
                e_t, rb16, h, first, last = pend
                if first:
                    wacc_tiles[h] = wp.tile([P, 512], F32, tag="w", name="wacc")
                    # single full-bank zero-matmul opens the accumulation:
                    # start=True clearing is coarser than a [4,128] region,
                    # so per-region start bits would wipe sibling regions.
                    nc.tensor.matmul(
                        wacc_tiles[h],
                        lhsT=zs128,
                        rhs=e_t[:, 0:512],
                        start=True,
                        stop=False,
                        skip_group_check=True,
                    )
                wps = wacc_tiles[h]
                for j in range(4):
                    for s in range(4):
                        nc.tensor.matmul(
                            wps[32 * s : 32 * s + 4, ts(j, P)],
                            lhsT=rb16[:, 4 * j : 4 * j + 4],
                            rhs=e_t[:, ds(512 * j + 128 * s, P)],
                            start=False,
                            stop=last,
                            tile_position=(0, 32 * s),
                            skip_group_check=True,
                        )

            pend_s = emit_S(0, 0)
            pend_w = None
            for gi in range(NH * QT_TILES):
                e_t = ep.tile([P, N], BF, tag="e", name="e_t")
                zs = []
                for kk, s_ps in enumerate(pend_s):
                    z_t = zp.tile([P, 1], F32, tag=f"z{kk}", name="z_t")
                    nc.scalar.activation(
                        out=e_t[:, ts(kk, 1024)],
                        in_=s_ps,
                        func=AF.Exp,
                        scale=inv_exp,
                        accum_out=z_t,
                    )
                    zs.append(z_t)
                if gi + 1 < NH * QT_TILES:
                    pend_s = emit_S((gi + 1) // QT_TILES, (gi + 1) % QT_TILES)
                r_t = zp.tile([P, 1], F32, tag="r", name="r_t")
                nc.vector.tensor_add(r_t, zs[0], zs[1])
                nc.vector.reciprocal(r_t, r_t)
                rb16 = zp.tile([P, 4 * NH], BF, tag="rb", name="rb16")
                nc.vector.tensor_tensor(
                    rb16,
                    mask16,
                    r_t.to_broadcast((P, 4 * NH)),
                    mybir.AluOpType.mult,
                )
                bg_advance(gi)
                if pend_w is not None:
                    emit_w(pend_w)
                    if pend_w[4]:  # closed head pend_w[2]'s accumulator
                        aux.append(finalize(pend_w[2]))
                pend_w = (
                    e_t, rb16, gi // QT_TILES,
                    gi % QT_TILES == 0, gi % QT_TILES == QT_TILES - 1,
                )
                if gi == 52:
                    aux.append(attend(0))
                elif gi == 56:
                    aux.append(attend(1))
                elif gi == 60:
                    aux.append(attend(2))
                aux_step(n=2)

            emit_w(pend_w)
            aux.append(finalize(3))
            aux.append(attend(3))
            aux_step(n=100)
            nc.vector.tensor_copy(pooled2_sb, pooled_tile[0])
            nc.sync.dma_start(
                out=out_d.rearrange("a (b c) -> (a b) c", b=2),
                in_=pooled2_sb,
            )

    nc.finalize()
    return nc


def _get_nc():
    if "nc" not in _cache:
        _cache["nc"] = _build_nc()
    return _cache["nc"]


def _f8(a):
    return np.clip(a, -240.0, 240.0).astype(F8)


def _host_prep(inputs):
    """Build the 8 per-core input maps (shard + transpose + quantize)."""
    x = np.asarray(inputs["chunk_embeddings"], np.float32)
    wq = np.asarray(inputs["Wq"], np.float32)
    wk = np.asarray(inputs["Wk"], np.float32)
    wv = np.asarray(inputs["Wv"], np.float32)
    wo = np.asarray(inputs["Wo"], np.float32)
    bq = np.asarray(inputs["bq"], np.float32)
    in_maps = []
    for c in range(NCORES):
        b, hg = c // 2, c % 2
        sl = slice(hg * HGW, (hg + 1) * HGW)
        xT = x[b].T  # (1024, 2048): [po*128+pi, n]
        # xq8[pi, c, po, n'] = x[b, c*512+n', po*128+pi]
        xq8 = _f8(
            np.ascontiguousarray(
                xT.reshape(8, P, NCHUNK, 512).transpose(1, 2, 0, 3)
            )
        )
        # w?8[pi, h, po, d] = 32*W[hg*512+h*128+d, po*128+pi]
        def w8(W):
            m = (32.0 * W[sl, :]).T.reshape(8, P, NH, P).transpose(1, 2, 0, 3)
            return _f8(np.ascontiguousarray(m))
        # xbf[pi, po, n]
        xbf = np.ascontiguousarray(xT.reshape(8, P, N).transpose(1, 0, 2)).astype(
            BF16
        )
        # wvb[pi, po, o] = Wv[hg*512+o, po*128+pi]
        wvb = np.ascontiguousarray(
            wv[sl, :].T.reshape(8, P, HGW).transpose(1, 0, 2)
        ).astype(BF16)
        # wob[pi, h, o] = Wo[o, hg*512+h*128+pi]
        wob = np.ascontiguousarray(
            wo[:, sl].T.reshape(NH, P, HID).transpose(1, 0, 2)
        ).astype(BF16)
        bq32 = np.ascontiguousarray((32.0 * bq[sl]).reshape(NH, P).T)
        in_maps.append(
            {
                "xq8": xq8,
                "wq8": w8(wq),
                "wk8": w8(wk),
                "xbf": xbf,
                "wvb": wvb,
                "wob": wob,
                "bq32": bq32,
            }
        )
    return in_maps


def _unshard(results, inputs):
    bo = np.asarray(inputs["bo"], np.float32)
    bv = np.asarray(inputs["bv"], np.float32)
    Wo = np.asarray(inputs["Wo"], np.float32)
    bv_wo = Wo @ bv  # exact fold of the V bias through the output projection
    out = np.zeros((B, HID), np.float32)
    for b in range(B):
        out[b] = (
            results[2 * b]["out_pooled"][0]
            + results[2 * b + 1]["out_pooled"][0]
            + bv_wo
            + bo
        )
    return out


def _reference_numpy(inputs):
    """Fallback for non-trivial attention masks (never hit for the spec'd
    all-ones mask): straight numpy port of the reference."""
    x = np.asarray(inputs["chunk_embeddings"], np.float32)
    mask = np.asarray(inputs["attention_mask"])
    b, n, hid = x.shape

    def proj(W, bias):
        y = x @ np.asarray(W, np.float32).T + np.asarray(bias, np.float32)
        return y.reshape(b, n, HEADS, HD).transpose(0, 2, 1, 3)

    Q = proj(inputs["Wq"], inputs["bq"])
    K = proj(inputs["Wk"], inputs["bk"])
    V = proj(inputs["Wv"], inputs["bv"])
    s = np.einsum("bhqd,bhkd->bhqk", Q, K) / np.float32(np.sqrt(HD))
    s = np.where(mask[:, None, None, :] == 0, np.float32(-1e9), s)
    s = s - s.max(axis=-1, keepdims=True)
    e = np.exp(s)
    a = e / e.sum(axis=-1, keepdims=True)
    att = np.einsum("bhqk,bhkd->bhqd", a, V)
    att = att.transpose(0, 2, 1, 3).reshape(b, n, hid)
    out = att @ np.asarray(inputs["Wo"], np.float32).T + np.asarray(
        inputs["bo"], np.float32
    )
    m = mask[:, :, None].astype(np.float32)
    return (out * m).sum(axis=1) / m.sum(axis=1)


def _run(inputs, trace=False):
    from concourse.bass_utils import run_bass_kernel_spmd

    nc = _get_nc()
    in_maps = _host_prep(inputs)
    res = run_bass_kernel_spmd(
        nc, in_maps, core_ids=list(range(NCORES)), trace=trace
    )
    _cache["last_result"] = res
    return _unshard(res.results, inputs)


def kernel(**inputs):
    mask = np.asarray(inputs["attention_mask"])
    if not np.all(mask == 1):
        return _reference_numpy(inputs)
    return _run(inputs, trace=False)


def kernel_traced(**inputs):
    """Like kernel() but with NTFF profiling; returns (out, exec_time_ns)."""
    out = _run(inputs, trace=True)
    return out, _cache["last_result"].exec_time_ns


# revision 51
# speedup vs baseline: 1.0143x; 1.0060x over previous
"""AttentionPooling Trainium2 kernel (8 NeuronCores, Bass/Tile).

Sharding: (batch, head-group) — core c handles batch b=c//2 and heads
4*(c%2)..4*(c%2)+3. Each core computes, for its 4 heads, Q^T/K^T (head-dim
major) projections and V (token major), then a one-pass pooled attention:

  For each query stripe of 128 rows:  S = Q_stripe K^T  (PE, bf16)
  E = exp(S/(1024*sqrt(d))) (ScalarE, accum_out -> Z), r = 1/Z (VectorE)
  wacc[j, c] += onehot_j(r)^T E[:, j*512+c]  (PE, per-head [4,512] PSUM
  accumulator; the [4,128] block m transposes to w columns of k-tile 4j+m)

  attended_h = sum_t wT[k-tile t]^T V[t]  (PE, one-hot [4,128] accumulator)
  pooled = concat_h(attended) @ Wo_slice^T / N   (folded mean-pool)

Numerics: Q/K projections run fp8(e4m3) x fp8 with weights pre-scaled by
32 on the host (the 1/1024 folds into the exp scale) — fp8 matmuls run at
bf16 speed, but halve the critical-path DMA bytes. V stays bf16 (fp8 Wv
error does NOT average out through w@V); host-verified max rel err 5.6e-3
vs the fp32 reference. The K bias is dropped: it only adds a per-query
constant to the scores, which softmax cancels. V/output biases fold on the
host: pooled += Wo@bv + bo.

Schedule: a dummy exp preloads the ACT table and a few junk matmuls warm
the PE clock-gate while the critical DMAs land (fp8 x + h0 weights; the
bf16 x / Wv / Wo loads are dependency-deferred behind the prologue so they
don't steal DMA bandwidth). K(h0)+Q(h0,c0) project as a prologue; the
remaining Q/K chunks and V tiles interleave between attention stripes via
a cost/deadline-paced background queue. Per-head w finalizes (transpose to
wT) while the next head's stripes run; attended matmuls are slotted at
stripes 49/53/57; only head 3's finalize+attend and the 8 pooled matmuls
trail the last stripe.

PSUM (8 banks): S stripes 2x[128,1024] (4) + projection chunks 2x[128,512]
(2, also lent to small finalize tiles) + w/attended accumulators 2x (2).
"""

import math
import sys

import numpy as np

for _p in ("/opt/trn_rl_repo",):
    if _p not in sys.path:
        sys.path.append(_p)

import ml_dtypes

B, N, HID = 4, 2048, 1024
HEADS, HD = 8, 128
NH = 4          # heads per core
HGW = NH * HD   # head-group width (512)
NCORES = 8
P = 128
QT_TILES = N // P    # 16 query stripes per head
TOK_TILES = N // P   # 16 token tiles
NCHUNK = 4           # 512-token projection chunks

BF16 = ml_dtypes.bfloat16
F8 = ml_dtypes.float8_e4m3  # TRN fp8e4: max 240

_cache = {}


def _build_nc():
    import concourse.bacc as bacc
    import concourse.tile as tile
    from concourse import mybir
    from concourse.bass import ds, ts
    from concourse.masks import make_identity
    from concourse.tile import add_dep_helper

    BF = mybir.dt.bfloat16
    F32 = mybir.dt.float32
    FP8 = mybir.dt.float8e4
    AF = mybir.ActivationFunctionType

    nc = bacc.Bacc(trn_type="TRN2")

    # fp8 x, token-chunk major: xq8[pi, c, po, n'] = x[b, c*512+n', po*128+pi]
    xq8_d = nc.dram_tensor("xq8", (P, NCHUNK, 8, 512), FP8, kind="ExternalInput").ap()
    # fp8 Q/K weights (x32): wq8[pi, h, po, d] = 32*Wq[hg*512+h*128+d, po*128+pi]
    wq8_d = nc.dram_tensor("wq8", (P, NH, 8, P), FP8, kind="ExternalInput").ap()
    wk8_d = nc.dram_tensor("wk8", (P, NH, 8, P), FP8, kind="ExternalInput").ap()
    # bf16 x, hid-tile major: xbf[pi, po, n] = x[b, n, po*128+pi]
    xbf_d = nc.dram_tensor("xbf", (P, 8, N), BF, kind="ExternalInput").ap()
    # bf16 V weights: wvb[pi, po, o] = Wv[hg*512+o, po*128+pi]
    wvb_d = nc.dram_tensor("wvb", (P, 8, HGW), BF, kind="ExternalInput").ap()
    # bf16 O weights: wob[pi, h, o] = Wo[o, hg*512+h*128+pi]
    wob_d = nc.dram_tensor("wob", (P, NH, HID), BF, kind="ExternalInput").ap()
    bq_d = nc.dram_tensor("bq32", (P, NH), F32, kind="ExternalInput").ap()
    out_d = nc.dram_tensor("out_pooled", (1, HID), F32, kind="ExternalOutput").ap()

    inv_exp = float(1.0 / (1024.0 * math.sqrt(HD)))
    inv_pool = float(1.0 / N)

    with tile.TileContext(nc) as tc:
        with (
            tc.tile_pool(name="persist", bufs=1) as persist,
            tc.tile_pool(name="sp", bufs=2, space="PSUM") as sp,
            tc.tile_pool(name="pp", bufs=2, space="PSUM") as pp,
            tc.tile_pool(name="wp", bufs=2, space="PSUM") as wp,
            tc.tile_pool(name="ep", bufs=3) as ep,
            tc.tile_pool(name="zp", bufs=4) as zp,
        ):
            # ---- input DMAs, emitted first so the queues start at t0 ----
            # Per-queue FIFO tiering (no dep-gating — dep-gated DMAs degrade
            # to descriptor-at-a-time trickle): tier 1 = K(h0,c0) operands
            # across all 16 queues, tier 2 = remaining token chunks, tier 3
            # = everything the background projections need later.
            xq8_sb = [
                persist.tile([P, 8, 512], FP8, name=f"xq8_{i}")
                for i in range(NCHUNK)
            ]
            wq8_sb = [
                persist.tile([P, 8, P], FP8, name=f"wq8_{i}") for i in range(NH)
            ]
            wk8_sb = [
                persist.tile([P, 8, P], FP8, name=f"wk8_{i}") for i in range(NH)
            ]
            xbf_sb = persist.tile([P, 8, N], BF)
            wvb_sb = persist.tile([P, 8, HGW], BF)
            wob_sb = persist.tile([P, NH, HID], BF)
            bq_sb = persist.tile([P, NH], F32)

            def dma_split(dst, src_, nsplit):
                step = P // nsplit
                for i in range(nsplit):
                    nc.sync.dma_start(
                        out=dst[i * step : (i + 1) * step],
                        in_=src_[i * step : (i + 1) * step],
                    )

            # Each dma_start costs ~0.6us of serial issue on the Sync queue
            # (DIRECT2D), so the count is minimized and ordered so K(h0,c0)'s
            # operands issue first; transfers overlap later issues.
            nc.sync.dma_start(out=wk8_sb[0], in_=wk8_d[:, 0])
            dma_split(xq8_sb[0], xq8_d[:, 0], 2)
            nc.sync.dma_start(out=wq8_sb[0], in_=wq8_d[:, 0])
            for c in range(1, NCHUNK):
                nc.sync.dma_start(out=xq8_sb[c], in_=xq8_d[:, c])
            nc.sync.dma_start(out=bq_sb, in_=bq_d)
            for h in range(1, NH):
                nc.sync.dma_start(out=wk8_sb[h], in_=wk8_d[:, h])
                nc.sync.dma_start(out=wq8_sb[h], in_=wq8_d[:, h])
            for half in range(2):
                nc.sync.dma_start(
                    out=xbf_sb[:, 4 * half : 4 * half + 4, :],
                    in_=xbf_d[:, 4 * half : 4 * half + 4, :],
                )
            nc.sync.dma_start(out=wvb_sb, in_=wvb_d)
            nc.sync.dma_start(out=wob_sb, in_=wob_d)

            # ---- small constants (DVE) --------------------------------
            # mask16 columns {0,5,10,15} are 1: slicing [:, 4j:4j+4] gives
            # the one-hot column j used to route r into wacc row j.
            mask16 = persist.tile([P, 4 * NH], BF)
            nc.vector.memset(mask16, 0.0)
            for j in range(4):
                nc.vector.memset(mask16[:, 5 * j : 5 * j + 1], 1.0)
            zs128 = persist.tile([P, P], BF)
            nc.vector.memset(zs128, 0.0)
            ident4 = persist.tile([4, 4], F32)
            make_identity(nc, ident4)
            # 4x4 identity replicated at each 32-partition row group, so the
            # block transposes of wacc (stationary at base partition 32s) use
            # a moving operand at the same base partition.
            ident4x = persist.tile([P, 4], F32)
            nc.vector.memset(ident4x, 0.0)
            for s in range(4):
                nc.sync.dma_start(out=ident4x[32 * s : 32 * s + 4, :], in_=ident4)

            QT_sb = persist.tile([P, NH, N], BF)
            KT_sb = persist.tile([P, NH, N], BF)
            V_sb = persist.tile([P, TOK_TILES, HGW], BF)
            wacc_sb = persist.tile([P, 512], F32)
            # wT[pi, h, j, s, :]: [128,4] stationary for k-tile t=4j+s of
            # head h — one-hot at column j by construction (the transpose of
            # the block-diagonal wacc region), so head h's attended matmuls
            # accumulate partials into row j of a [4,128] PSUM tile.
            wT_sb = persist.tile([P, NH, 4, 4, 4], BF)
            att4_sb = persist.tile([4, P], F32)
            # attT2[:, h, oc, :]: [128,2] stationary with attended_h at
            # column oc (other column zero) so the two pooled-projection
            # matmuls of head h land in rows 0/1 of one [2,512] accumulator.
            attT2_sb = persist.tile([P, NH, 2, 2], BF)
            nc.vector.memset(attT2_sb, 0.0)
            pooled2_sb = persist.tile([2, 512], F32)

            # ---- ACT table preload + PE warmup (run under the DMAs) ---
            zdum = zp.tile([P, 16], BF, tag="zd", name="zdum")
            nc.scalar.activation(out=zdum, in_=mask16, func=AF.Exp)
            for _ in range(12):
                warm_ps = pp.tile([16, 512], F32, tag="proj", name="warm_ps")
                nc.tensor.matmul(
                    warm_ps, lhsT=mask16, rhs=KT_sb[:, 0, 0:512],
                    start=True, stop=True, skip_group_check=True,
                )

            # ---- projection emitters ----------------------------------
            def qk_chunk(proj_i, h, c, step=False):
                """512-token fp8 DoubleRow Q^T/K^T projection for head h:
                vitile v contracts hid pair-blocks (2v, 2v+1)."""
                wsb, dst = ((wq8_sb, QT_sb), (wk8_sb, KT_sb))[proj_i]
                ps = pp.tile([P, 512], F32, tag="proj", name="ps_qk")
                for v in range(4):
                    nc.tensor.matmul(
                        ps,
                        lhsT=wsb[h][:, 2 * v : 2 * v + 2, :],
                        rhs=xq8_sb[c][:, 2 * v : 2 * v + 2, :],
                        start=(v == 0),
                        stop=(v == 3),
                        perf_mode=mybir.MatmulPerfMode.DoubleRow,
                    )
                    if step and v == 1:
                        yield
                if proj_i == 0:
                    # Q bias (32*bq) folded into the psum->bf16 evacuation
                    ev = nc.vector.tensor_tensor(
                        dst[:, h, ts(c, 512)],
                        ps,
                        bq_sb[:, h : h + 1].to_broadcast((P, 512)),
                        mybir.AluOpType.add,
                    )
                else:
                    ev = nc.vector.tensor_copy(dst[:, h, ts(c, 512)], ps)
                if step:
                    yield ev

            def v_chunk(t, step=False):
                """128-token bf16 V projection tile (all 4 heads)."""
                ps = pp.tile([P, HGW], F32, tag="proj", name="ps_v")
                for i in range(8):
                    nc.tensor.matmul(
                        ps,
                        lhsT=xbf_sb[:, i, ts(t, P)],
                        rhs=wvb_sb[:, i, :],
                        start=(i == 0),
                        stop=(i == 7),
                    )
                    if step and i in (2, 5):
                        yield
                nc.vector.tensor_copy(V_sb[:, t, :], ps)
                if step:
                    yield

            # ---- prologue: K(h0) + Q(h0,c0) ---------------------------
            for c in range(NCHUNK):
                for _ in qk_chunk(1, 0, c):
                    pass
            for _ in qk_chunk(0, 0, 0):
                pass

            # ---- background queue: (generator, est_ns, deadline, nb) --
            bg = []
            for c in range(1, NCHUNK):
                bg.append((qk_chunk(0, 0, c, True), 1100.0, 4 * c - 2, 0))
            for h in range(1, NH):
                for c in range(NCHUNK):
                    bg.append((qk_chunk(1, h, c, True), 1100.0, 16 * h - 4 + c, 0))
                for c in range(NCHUNK):
                    bg.append(
                        (qk_chunk(0, h, c, True), 1100.0, 16 * h + 4 * c - 2, 0)
                    )
            for t in range(TOK_TILES):
                bg.append((v_chunk(t, True), 3000.0, 33 + t, 14 + t))
            bg_total = sum(u[1] for u in bg)
            bg_state = {"i": 0, "spent": 0.0}
            BG_SPREAD = 52  # finish all background work by stripe 52 of 64

            def bg_step():
                gen, cost, _, _ = bg[bg_state["i"]]
                try:
                    next(gen)
                    bg_state["spent"] += cost / 3.0
                except StopIteration:
                    bg_state["spent"] = sum(u[1] for u in bg[: bg_state["i"] + 1])
                    bg_state["i"] += 1

            def bg_advance(si):
                while bg_state["i"] < len(bg) and bg[bg_state["i"]][2] <= si + 1:
                    bg_step()
                target = (si + 1) * bg_total / BG_SPREAD
                while (
                    bg_state["i"] < len(bg)
                    and bg_state["spent"] < target
                    and bg[bg_state["i"]][3] <= si
                ):
                    bg_step()

            # ---- per-head finalize + attended (aux-paced) -------------
            wacc_tiles = {}
            pooled_tile = [None]

            def finalize(h):
                """wacc (PSUM, block-diag) -> wT_sb[:, h] one-hot k-tiles.
                The scale-copy (first step) releases the wacc pool slot; the
                16 transposes spread over the following stripes."""
                wps = wacc_tiles.pop(h)
                nc.vector.tensor_scalar_mul(wacc_sb, wps, inv_pool)
                yield
                for s in range(4):
                    for j in range(4):
                        tp = pp.tile([P, 4], F32, tag="proj", name="tp_ps")
                        nc.tensor.transpose(
                            tp,
                            wacc_sb[32 * s : 32 * s + 4, ts(j, P)],
                            ident4x[32 * s : 32 * s + 4, :],
                            tile_position=(32 * s, 0),
                        )
                        nc.vector.tensor_copy(wT_sb[:, h, j, s, :], tp)
                    yield

            def attend(h):
                """attended_h = sum_t wT[k-tile t]^T V[t, head h], then its
                two pooled-projection matmuls into the shared accumulator."""
                aps = pp.tile([4, P], F32, tag="proj", name="att4_ps")
                for t in range(TOK_TILES):
                    nc.tensor.matmul(
                        aps,
                        lhsT=wT_sb[:, h, t // 4, t % 4, :],
                        rhs=V_sb[:, t, ts(h, HD)],
                        start=(t == 0),
                        stop=(t == TOK_TILES - 1),
                    )
                    if t in (3, 7, 11):
                        yield
                nc.vector.tensor_copy(att4_sb, aps)
                atp = pp.tile([P, 4], F32, tag="proj", name="attT_ps")
                nc.tensor.transpose(atp, att4_sb, ident4)
                ar = zp.tile([P, 1], F32, tag="ar", name="attr")
                nc.vector.reduce_sum(ar, atp, axis=mybir.AxisListType.X)
                for oc in range(2):
                    nc.vector.tensor_copy(attT2_sb[:, h, oc, oc : oc + 1], ar)
                if pooled_tile[0] is None:
                    pooled_tile[0] = wp.tile([2, 512], F32, tag="w", name="pooled")
                for oc in range(2):
                    nc.tensor.matmul(
                        pooled_tile[0],
                        lhsT=attT2_sb[:, h, oc, :],
                        rhs=wob_sb[:, h, ts(oc, 512)],
                        start=(h == 0 and oc == 0),
                        stop=(h == NH - 1 and oc == 1),
                        skip_group_check=True,
                    )
                yield

            aux = []

            def aux_step(n=1):
                for _ in range(n):
                    while aux:
                        try:
                            next(aux[0])
                            break
                        except StopIteration:
                            aux.pop(0)

            # ---- pooled attention stripe loop -------------------------
            def emit_S(h, qi):
                tiles = []
                for kk in range(2):
                    s_ps = sp.tile([P, 1024], F32, tag="s", name="s_ps")
                    for kc in range(2):
                        nc.tensor.matmul(
                            s_ps[:, ts(kc, 512)],
                            lhsT=QT_sb[:, h, ts(qi, P)],
                            rhs=KT_sb[:, h, ds(kk * 1024 + kc * 512, 512)],
                            start=True,
                            stop=True,
                        )
                    tiles.append(s_ps)
                return tiles

            def emit_w(pend):
                # 16 [4,128] matmuls, 4-way col-group concurrent: region
                # (j, s) at partitions [32s, 32s+4), free [128j, 128j+128)
                # holds w[j*512+s*128+c] at row j (one-hot lhsT), i.e. the
                # [4,128] block (s, j) transposes to k-tile 4j+s.
                e_t, rb16, h, first, last = pend
                if first:
                    wacc_tiles[h] = wp.tile([P, 512], F32, tag="w", name="wacc")
                    # single full-bank zero-matmul opens the accumulation:
                    # start=True clearing is coarser than a [4,128] region,
                    # so per-region start bits would wipe sibling regions.
                    nc.tensor.matmul(
                        wacc_tiles[h],
                        lhsT=zs128,
                        rhs=e_t[:, 0:512],
                        start=True,
                        stop=False,
                        skip_group_check=True,
                    )
                wps = wacc_tiles[h]
                for j in range(4):
                    for s in range(4):
                        nc.tensor.matmul(
                            wps[32 * s : 32 * s + 4, ts(j, P)],
                            lhsT=rb16[:, 4 * j : 4 * j + 4],
                            rhs=e_t[:, ds(512 * j + 128 * s, P)],
                            start=False,
                            stop=last,
                            tile_position=(0, 32 * s),
                            skip_group_check=True,
                        )

            pend_s = emit_S(0, 0)
            pend_w = None
            for gi in range(NH * QT_TILES):
                e_t = ep.tile([P, N], BF, tag="e", name="e_t")
                zs = []
                for kk, s_ps in enumerate(pend_s):
                    z_t = zp.tile([P, 1], F32, tag=f"z{kk}", name="z_t")
                    nc.scalar.activation(
                        out=e_t[:, ts(kk, 1024)],
                        in_=s_ps,
                        func=AF.Exp,
                        scale=inv_exp,
                        accum_out=z_t,
                    )
                    zs.append(z_t)
                if gi + 1 < NH * QT_TILES:
                    pend_s = emit_S((gi + 1) // QT_TILES, (gi + 1) % QT_TILES)
                r_t = zp.tile([P, 1], F32, tag="r", name="r_t")
                nc.vector.tensor_add(r_t, zs[0], zs[1])
                nc.vector.reciprocal(r_t, r_t)
                rb16 = zp.tile([P, 4 * NH], BF, tag="rb", name="rb16")
                nc.vector.tensor_tensor(
                    rb16,
                    mask16,
                    r_t.to_broadcast((P, 4 * NH)),
                    mybir.AluOpType.mult,
                )
                bg_advance(gi)
                if pend_w is not None:
                    emit_w(pend_w)
                    if pend_w[4]:  # closed head pend_w[2]'s accumulator
                        aux.append(finalize(pend_w[2]))
                pend_w = (
                    e_t, rb16, gi // QT_TILES,
                    gi % QT_TILES == 0, gi % QT_TILES == QT_TILES - 1,
                )
                if gi == 52:
                    aux.append(attend(0))
                elif gi == 56:
                    aux.append(attend(1))
                elif gi == 60:
                    aux.append(attend(2))
                aux_step()

            emit_w(pend_w)
            aux.append(finalize(3))
            aux.append(attend(3))
            aux_step(n=100)
            nc.vector.tensor_copy(pooled2_sb, pooled_tile[0])
            nc.sync.dma_start(
                out=out_d.rearrange("a (b c) -> (a b) c", b=2),
                in_=pooled2_sb,
            )

    nc.finalize()
    return nc


def _get_nc():
    if "nc" not in _cache:
        _cache["nc"] = _build_nc()
    return _cache["nc"]


def _f8(a):
    return np.clip(a, -240.0, 240.0).astype(F8)


def _host_prep(inputs):
    """Build the 8 per-core input maps (shard + transpose + quantize)."""
    x = np.asarray(inputs["chunk_embeddings"], np.float32)
    wq = np.asarray(inputs["Wq"], np.float32)
    wk = np.asarray(inputs["Wk"], np.float32)
    wv = np.asarray(inputs["Wv"], np.float32)
    wo = np.asarray(inputs["Wo"], np.float32)
    bq = np.asarray(inputs["bq"], np.float32)
    in_maps = []
    for c in range(NCORES):
        b, hg = c // 2, c % 2
        sl = slice(hg * HGW, (hg + 1) * HGW)
        xT = x[b].T  # (1024, 2048): [po*128+pi, n]
        # xq8[pi, c, po, n'] = x[b, c*512+n', po*128+pi]
        xq8 = _f8(
            np.ascontiguousarray(
                xT.reshape(8, P, NCHUNK, 512).transpose(1, 2, 0, 3)
            )
        )
        # w?8[pi, h, po, d] = 32*W[hg*512+h*128+d, po*128+pi]
        def w8(W):
            m = (32.0 * W[sl, :]).T.reshape(8, P, NH, P).transpose(1, 2, 0, 3)
            return _f8(np.ascontiguousarray(m))
        # xbf[pi, po, n]
        xbf = np.ascontiguousarray(xT.reshape(8, P, N).transpose(1, 0, 2)).astype(
            BF16
        )
        # wvb[pi, po, o] = Wv[hg*512+o, po*128+pi]
        wvb = np.ascontiguousarray(
            wv[sl, :].T.reshape(8, P, HGW).transpose(1, 0, 2)
        ).astype(BF16)
        # wob[pi, h, o] = Wo[o, hg*512+h*128+pi]
        wob = np.ascontiguousarray(
            wo[:, sl].T.reshape(NH, P, HID).transpose(1, 0, 2)
        ).astype(BF16)
        bq32 = np.ascontiguousarray((32.0 * bq[sl]).reshape(NH, P).T)
        in_maps.append(
            {
                "xq8": xq8,
                "wq8": w8(wq),
                "wk8": w8(wk),
                "xbf": xbf,
                "wvb": wvb,
                "wob": wob,
                "bq32": bq32,
            }
        )
    return in_maps


def _unshard(results, inputs):
    bo = np.asarray(inputs["bo"], np.float32)
    bv = np.asarray(inputs["bv"], np.float32)
    Wo = np.asarray(inputs["Wo"], np.float32)
    bv_wo = Wo @ bv  # exact fold of the V bias through the output projection
    out = np.zeros((B, HID), np.float32)
    for b in range(B):
        out[b] = (
            results[2 * b]["out_pooled"][0]
            + results[2 * b + 1]["out_pooled"][0]
            + bv_wo
            + bo
        )
    return out


def _reference_numpy(inputs):
    """Fallback for non-trivial attention masks (never hit for the spec'd
    all-ones mask): straight numpy port of the reference."""
    x = np.asarray(inputs["chunk_embeddings"], np.float32)
    mask = np.asarray(inputs["attention_mask"])
    b, n, hid = x.shape

    def proj(W, bias):
        y = x @ np.asarray(W, np.float32).T + np.asarray(bias, np.float32)
        return y.reshape(b, n, HEADS, HD).transpose(0, 2, 1, 3)

    Q = proj(inputs["Wq"], inputs["bq"])
    K = proj(inputs["Wk"], inputs["bk"])
    V = proj(inputs["Wv"], inputs["bv"])
    s = np.einsum("bhqd,bhkd->bhqk", Q, K) / np.float32(np.sqrt(HD))
    s = np.where(mask[:, None, None, :] == 0, np.float32(-1e9), s)
    s = s - s.max(axis=-1, keepdims=True)
    e = np.exp(s)
    a = e / e.sum(axis=-1, keepdims=True)
    att = np.einsum("bhqk,bhkd->bhqd", a, V)
    att = att.transpose(0, 2, 1, 3).reshape(b, n, hid)
    out = att @ np.asarray(inputs["Wo"], np.float32).T + np.asarray(
        inputs["bo"], np.float32
    )
    m = mask[:, :, None].astype(np.float32)
    return (out * m).sum(axis=1) / m.sum(axis=1)


def _run(inputs, trace=False):
    from concourse.bass_utils import run_bass_kernel_spmd

    nc = _get_nc()
    in_maps = _host_prep(inputs)
    res = run_bass_kernel_spmd(
        nc, in_maps, core_ids=list(range(NCORES)), trace=trace
    )
    _cache["last_result"] = res
    return _unshard(res.results, inputs)


def kernel(**inputs):
    mask = np.asarray(inputs["attention_mask"])
    if not np.all(mask == 1):
        return _reference_numpy(inputs)
    return _run(inputs, trace=False)


def kernel_traced(**inputs):
    """Like kernel() but with NTFF profiling; returns (out, exec_time_ns)."""
    out = _run(inputs, trace=True)
    return out, _cache["last_result"].exec_time_ns


# revision 52
# speedup vs baseline: 1.1754x; 1.1587x over previous
"""AttentionPooling Trainium2 kernel (8 NeuronCores, Bass/Tile).

Sharding: (batch, head-group) — core c handles batch b=c//2 and heads
4*(c%2)..4*(c%2)+3. Each core computes, for its 4 heads, Q^T/K^T (head-dim
major) projections and V (token major), then a one-pass pooled attention:

  For each query stripe of 128 rows:  S = Q_stripe K^T  (PE, bf16)
  E = exp(S/(1024*sqrt(d))) (ScalarE, accum_out -> Z), r = 1/Z (VectorE)
  wacc[j, c] += onehot_j(r)^T E[:, j*512+c]  (PE, per-head [4,512] PSUM
  accumulator; the [4,128] block m transposes to w columns of k-tile 4j+m)

  attended_h = sum_t wT[k-tile t]^T V[t]  (PE, one-hot [4,128] accumulator)
  pooled = concat_h(attended) @ Wo_slice^T / N   (folded mean-pool)

Numerics: Q/K projections run fp8(e4m3) x fp8 with weights pre-scaled by
32 on the host (the 1/1024 folds into the exp scale) — fp8 matmuls run at
bf16 speed, but halve the critical-path DMA bytes. V stays bf16 (fp8 Wv
error does NOT average out through w@V); host-verified max rel err 5.6e-3
vs the fp32 reference. The K bias is dropped: it only adds a per-query
constant to the scores, which softmax cancels. V/output biases fold on the
host: pooled += Wo@bv + bo.

Schedule: a dummy exp preloads the ACT table and a few junk matmuls warm
the PE clock-gate while the critical DMAs land (fp8 x + h0 weights; the
bf16 x / Wv / Wo loads are dependency-deferred behind the prologue so they
don't steal DMA bandwidth). K(h0)+Q(h0,c0) project as a prologue; the
remaining Q/K chunks and V tiles interleave between attention stripes via
a cost/deadline-paced background queue. Per-head w finalizes (transpose to
wT) while the next head's stripes run; attended matmuls are slotted at
stripes 49/53/57; only head 3's finalize+attend and the 8 pooled matmuls
trail the last stripe.

PSUM (8 banks): S stripes 2x[128,1024] (4) + projection chunks 2x[128,512]
(2, also lent to small finalize tiles) + w/attended accumulators 2x (2).
"""

import math
import sys

import numpy as np

for _p in ("/opt/trn_rl_repo",):
    if _p not in sys.path:
        sys.path.append(_p)

import ml_dtypes

B, N, HID = 4, 2048, 1024
HEADS, HD = 8, 128
NH = 4          # heads per core
HGW = NH * HD   # head-group width (512)
NCORES = 8
P = 128
QT_TILES = N // P    # 16 query stripes per head
TOK_TILES = N // P   # 16 token tiles
NCHUNK = 4           # 512-token projection chunks

BF16 = ml_dtypes.bfloat16
F8 = ml_dtypes.float8_e4m3  # TRN fp8e4: max 240

_cache = {}


def _build_nc():
    import concourse.bacc as bacc
    import concourse.tile as tile
    from concourse import mybir
    from concourse.bass import ds, ts
    from concourse.masks import make_identity
    from concourse.tile import add_dep_helper

    BF = mybir.dt.bfloat16
    F32 = mybir.dt.float32
    FP8 = mybir.dt.float8e4
    AF = mybir.ActivationFunctionType

    nc = bacc.Bacc(trn_type="TRN2")

    # fp8 x, token-chunk major: xq8[pi, c, po, n'] = x[b, c*512+n', po*128+pi]
    xq8_d = nc.dram_tensor("xq8", (P, NCHUNK, 8, 512), FP8, kind="ExternalInput").ap()
    # fp8 Q/K weights (x32): wq8[pi, h, po, d] = 32*Wq[hg*512+h*128+d, po*128+pi]
    wq8_d = nc.dram_tensor("wq8", (P, NH, 8, P), FP8, kind="ExternalInput").ap()
    wk8_d = nc.dram_tensor("wk8", (P, NH, 8, P), FP8, kind="ExternalInput").ap()
    # bf16 x, hid-tile major: xbf[pi, po, n] = x[b, n, po*128+pi]
    xbf_d = nc.dram_tensor("xbf", (P, 8, N), BF, kind="ExternalInput").ap()
    # bf16 V weights: wvb[pi, po, o] = Wv[hg*512+o, po*128+pi]
    wvb_d = nc.dram_tensor("wvb", (P, 8, HGW), BF, kind="ExternalInput").ap()
    # bf16 O weights: wob[pi, h, o] = Wo[o, hg*512+h*128+pi]
    wob_d = nc.dram_tensor("wob", (P, NH, HID), BF, kind="ExternalInput").ap()
    bq_d = nc.dram_tensor("bq32", (P, NH), F32, kind="ExternalInput").ap()
    out_d = nc.dram_tensor("out_pooled", (1, HID), F32, kind="ExternalOutput").ap()

    inv_exp = float(1.0 / (1024.0 * math.sqrt(HD)))
    inv_pool = float(1.0 / N)

    with tile.TileContext(nc) as tc:
        with (
            tc.tile_pool(name="persist", bufs=1) as persist,
            tc.tile_pool(name="sp", bufs=2, space="PSUM") as sp,
            tc.tile_pool(name="pp", bufs=2, space="PSUM") as pp,
            tc.tile_pool(name="wp", bufs=2, space="PSUM") as wp,
            tc.tile_pool(name="ep", bufs=3) as ep,
            tc.tile_pool(name="zp", bufs=4) as zp,
        ):
            # ---- input DMAs, emitted first so the queues start at t0 ----
            # Per-queue FIFO tiering (no dep-gating — dep-gated DMAs degrade
            # to descriptor-at-a-time trickle): tier 1 = K(h0,c0) operands
            # across all 16 queues, tier 2 = remaining token chunks, tier 3
            # = everything the background projections need later.
            xq8_sb = [
                persist.tile([P, 8, 512], FP8, name=f"xq8_{i}")
                for i in range(NCHUNK)
            ]
            wq8_sb = [
                persist.tile([P, 8, P], FP8, name=f"wq8_{i}") for i in range(NH)
            ]
            wk8_sb = [
                persist.tile([P, 8, P], FP8, name=f"wk8_{i}") for i in range(NH)
            ]
            xbf_sb = persist.tile([P, 8, N], BF)
            wvb_sb = persist.tile([P, 8, HGW], BF)
            wob_sb = persist.tile([P, NH, HID], BF)
            bq_sb = persist.tile([P, NH], F32)

            def dma_split(dst, src_, nsplit):
                step = P // nsplit
                for i in range(nsplit):
                    nc.sync.dma_start(
                        out=dst[i * step : (i + 1) * step],
                        in_=src_[i * step : (i + 1) * step],
                    )

            # Each dma_start costs ~0.6us of serial issue on the Sync queue
            # (DIRECT2D), so the count is minimized and ordered so K(h0,c0)'s
            # operands issue first; transfers overlap later issues.
            nc.sync.dma_start(out=wk8_sb[0], in_=wk8_d[:, 0])
            dma_split(xq8_sb[0], xq8_d[:, 0], 2)
            nc.sync.dma_start(out=wq8_sb[0], in_=wq8_d[:, 0])
            for c in range(1, NCHUNK):
                nc.sync.dma_start(out=xq8_sb[c], in_=xq8_d[:, c])
            nc.sync.dma_start(out=bq_sb, in_=bq_d)
            for h in range(1, NH):
                nc.sync.dma_start(out=wk8_sb[h], in_=wk8_d[:, h])
                nc.sync.dma_start(out=wq8_sb[h], in_=wq8_d[:, h])
            for half in range(2):
                nc.sync.dma_start(
                    out=xbf_sb[:, 4 * half : 4 * half + 4, :],
                    in_=xbf_d[:, 4 * half : 4 * half + 4, :],
                )
            nc.sync.dma_start(out=wvb_sb, in_=wvb_d)
            nc.sync.dma_start(out=wob_sb, in_=wob_d)

            # ---- small constants (DVE) --------------------------------
            # mask16 columns {0,5,10,15} are 1: slicing [:, 4j:4j+4] gives
            # the one-hot column j used to route r into wacc row j.
            mask16 = persist.tile([P, 4 * NH], BF)
            nc.vector.memset(mask16, 0.0)
            for j in range(4):
                nc.vector.memset(mask16[:, 5 * j : 5 * j + 1], 1.0)
            zs128 = persist.tile([P, P], BF)
            nc.vector.memset(zs128, 0.0)
            ident4 = persist.tile([4, 4], F32)
            make_identity(nc, ident4)
            # 4x4 identity replicated at each 32-partition row group, so the
            # block transposes of wacc (stationary at base partition 32s) use
            # a moving operand at the same base partition.
            ident4x = persist.tile([P, 4], F32)
            nc.vector.memset(ident4x, 0.0)
            for s in range(4):
                nc.sync.dma_start(out=ident4x[32 * s : 32 * s + 4, :], in_=ident4)

            QT_sb = persist.tile([P, NH, N], BF)
            KT_sb = persist.tile([P, NH, N], BF)
            V_sb = persist.tile([P, TOK_TILES, HGW], BF)
            wacc_sb = persist.tile([P, 512], F32)
            # wT[pi, h, j, s, :]: [128,4] stationary for k-tile t=4j+s of
            # head h — one-hot at column j by construction (the transpose of
            # the block-diagonal wacc region), so head h's attended matmuls
            # accumulate partials into row j of a [4,128] PSUM tile.
            wT_sb = persist.tile([P, NH, 4, 4, 4], BF)
            att4_sb = persist.tile([4, P], F32)
            # attT2[:, h, oc, :]: [128,2] stationary with attended_h at
            # column oc (other column zero) so the two pooled-projection
            # matmuls of head h land in rows 0/1 of one [2,512] accumulator.
            attT2_sb = persist.tile([P, NH, 2, 2], BF)
            nc.vector.memset(attT2_sb, 0.0)
            pooled2_sb = persist.tile([2, 512], F32)

            # ---- ACT table preload + PE warmup (run under the DMAs) ---
            zdum = zp.tile([P, 16], BF, tag="zd", name="zdum")
            nc.scalar.activation(out=zdum, in_=mask16, func=AF.Exp)
            for _ in range(8):
                warm_ps = pp.tile([16, 512], F32, tag="proj", name="warm_ps")
                nc.tensor.matmul(
                    warm_ps, lhsT=mask16, rhs=KT_sb[:, 0, 0:512],
                    start=True, stop=True, skip_group_check=True,
                )

            # ---- projection emitters ----------------------------------
            def qk_chunk(proj_i, h, c, step=False):
                """512-token fp8 DoubleRow Q^T/K^T projection for head h:
                vitile v contracts hid pair-blocks (2v, 2v+1)."""
                wsb, dst = ((wq8_sb, QT_sb), (wk8_sb, KT_sb))[proj_i]
                ps = pp.tile([P, 512], F32, tag="proj", name="ps_qk")
                for v in range(4):
                    nc.tensor.matmul(
                        ps,
                        lhsT=wsb[h][:, 2 * v : 2 * v + 2, :],
                        rhs=xq8_sb[c][:, 2 * v : 2 * v + 2, :],
                        start=(v == 0),
                        stop=(v == 3),
                        perf_mode=mybir.MatmulPerfMode.DoubleRow,
                    )
                    if step and v == 1:
                        yield
                if proj_i == 0:
                    # Q bias (32*bq) folded into the psum->bf16 evacuation
                    ev = nc.vector.tensor_tensor(
                        dst[:, h, ts(c, 512)],
                        ps,
                        bq_sb[:, h : h + 1].to_broadcast((P, 512)),
                        mybir.AluOpType.add,
                    )
                else:
                    ev = nc.vector.tensor_copy(dst[:, h, ts(c, 512)], ps)
                if step:
                    yield ev

            def v_chunk(t, step=False):
                """128-token bf16 V projection tile (all 4 heads)."""
                ps = pp.tile([P, HGW], F32, tag="proj", name="ps_v")
                for i in range(8):
                    nc.tensor.matmul(
                        ps,
                        lhsT=xbf_sb[:, i, ts(t, P)],
                        rhs=wvb_sb[:, i, :],
                        start=(i == 0),
                        stop=(i == 7),
                    )
                    if step and i in (2, 5):
                        yield
                nc.vector.tensor_copy(V_sb[:, t, :], ps)
                if step:
                    yield

            # ---- prologue: K(h0) + Q(h0,c0) ---------------------------
            for c in range(NCHUNK):
                for _ in qk_chunk(1, 0, c):
                    pass
            for _ in qk_chunk(0, 0, 0):
                pass

            # ---- background queue: (generator, est_ns, deadline, nb) --
            bg = []
            for c in range(1, NCHUNK):
                bg.append((qk_chunk(0, 0, c, True), 1100.0, 4 * c - 2, 0))
            for h in range(1, NH):
                for c in range(NCHUNK):
                    bg.append((qk_chunk(1, h, c, True), 1100.0, 16 * h - 4 + c, 0))
                for c in range(NCHUNK):
                    bg.append(
                        (qk_chunk(0, h, c, True), 1100.0, 16 * h + 4 * c - 2, 0)
                    )
            for t in range(TOK_TILES):
                bg.append((v_chunk(t, True), 3000.0, 33 + t, 14 + t))
            bg_total = sum(u[1] for u in bg)
            bg_state = {"i": 0, "spent": 0.0}
            BG_SPREAD = 52  # finish all background work by stripe 52 of 64

            def bg_step():
                gen, cost, _, _ = bg[bg_state["i"]]
                try:
                    next(gen)
                    bg_state["spent"] += cost / 3.0
                except StopIteration:
                    bg_state["spent"] = sum(u[1] for u in bg[: bg_state["i"] + 1])
                    bg_state["i"] += 1

            def bg_advance(si):
                while bg_state["i"] < len(bg) and bg[bg_state["i"]][2] <= si + 1:
                    bg_step()
                target = (si + 1) * bg_total / BG_SPREAD
                while (
                    bg_state["i"] < len(bg)
                    and bg_state["spent"] < target
                    and bg[bg_state["i"]][3] <= si
                ):
                    bg_step()

            # ---- per-head finalize + attended (aux-paced) -------------
            wacc_tiles = {}
            pooled_tile = [None]

            def finalize(h):
                """wacc (PSUM, block-diag) -> wT_sb[:, h] one-hot k-tiles.
                The scale-copy (first step) releases the wacc pool slot; the
                16 transposes spread over the following stripes."""
                wps = wacc_tiles.pop(h)
                nc.vector.tensor_scalar_mul(wacc_sb, wps, inv_pool)
                yield
                for s in range(4):
                    for j in range(4):
                        tp = pp.tile([P, 4], F32, tag="proj", name="tp_ps")
                        nc.tensor.transpose(
                            tp,
                            wacc_sb[32 * s : 32 * s + 4, ts(j, P)],
                            ident4x[32 * s : 32 * s + 4, :],
                            tile_position=(32 * s, 0),
                        )
                        nc.vector.tensor_copy(wT_sb[:, h, j, s, :], tp)
                    yield

            def attend(h):
                """attended_h = sum_t wT[k-tile t]^T V[t, head h], then its
                two pooled-projection matmuls into the shared accumulator."""
                aps = pp.tile([4, P], F32, tag="proj", name="att4_ps")
                for t in range(TOK_TILES):
                    nc.tensor.matmul(
                        aps,
                        lhsT=wT_sb[:, h, t // 4, t % 4, :],
                        rhs=V_sb[:, t, ts(h, HD)],
                        start=(t == 0),
                        stop=(t == TOK_TILES - 1),
                    )
                    if t in (3, 7, 11):
                        yield
                nc.vector.tensor_copy(att4_sb, aps)
                atp = pp.tile([P, 4], F32, tag="proj", name="attT_ps")
                nc.tensor.transpose(atp, att4_sb, ident4)
                ar = zp.tile([P, 1], F32, tag="ar", name="attr")
                nc.vector.reduce_sum(ar, atp, axis=mybir.AxisListType.X)
                for oc in range(2):
                    nc.vector.tensor_copy(attT2_sb[:, h, oc, oc : oc + 1], ar)
                if pooled_tile[0] is None:
                    pooled_tile[0] = wp.tile([2, 512], F32, tag="w", name="pooled")
                for oc in range(2):
                    nc.tensor.matmul(
                        pooled_tile[0],
                        lhsT=attT2_sb[:, h, oc, :],
                        rhs=wob_sb[:, h, ts(oc, 512)],
                        start=(h == 0 and oc == 0),
                        stop=(h == NH - 1 and oc == 1),
                        skip_group_check=True,
                    )
                yield

            aux = []

            def aux_step(n=1):
                for _ in range(n):
                    while aux:
                        try:
                            next(aux[0])
                            break
                        except StopIteration:
                            aux.pop(0)

            # ---- pooled attention stripe loop -------------------------
            def emit_S(h, qi):
                tiles = []
                for kk in range(2):
                    s_ps = sp.tile([P, 1024], F32, tag="s", name="s_ps")
                    for kc in range(2):
                        nc.tensor.matmul(
                            s_ps[:, ts(kc, 512)],
                            lhsT=QT_sb[:, h, ts(qi, P)],
                            rhs=KT_sb[:, h, ds(kk * 1024 + kc * 512, 512)],
                            start=True,
                            stop=True,
                        )
                    tiles.append(s_ps)
                return tiles

            def emit_w(pend):
                # 16 [4,128] matmuls, 4-way col-group concurrent: region
                # (j, s) at partitions [32s, 32s+4), free [128j, 128j+128)
                # holds w[j*512+s*128+c] at row j (one-hot lhsT), i.e. the
                # [4,128] block (s, j) transposes to k-tile 4j+s.
                e_t, rb16, h, first, last = pend
                if first:
                    wacc_tiles[h] = wp.tile([P, 512], F32, tag="w", name="wacc")
                    # single full-bank zero-matmul opens the accumulation:
                    # start=True clearing is coarser than a [4,128] region,
                    # so per-region start bits would wipe sibling regions.
                    nc.tensor.matmul(
                        wacc_tiles[h],
                        lhsT=zs128,
                        rhs=e_t[:, 0:512],
                        start=True,
                        stop=False,
                        skip_group_check=True,
                    )
                wps = wacc_tiles[h]
                for j in range(4):
                    for s in range(4):
                        nc.tensor.matmul(
                            wps[32 * s : 32 * s + 4, ts(j, P)],
                            lhsT=rb16[:, 4 * j : 4 * j + 4],
                            rhs=e_t[:, ds(512 * j + 128 * s, P)],
                            start=False,
                            stop=last,
                            tile_position=(0, 32 * s),
                            skip_group_check=True,
                        )

            pend_s = emit_S(0, 0)
            pend_w = None
            for gi in range(NH * QT_TILES):
                e_t = ep.tile([P, N], BF, tag="e", name="e_t")
                zs = []
                for kk, s_ps in enumerate(pend_s):
                    z_t = zp.tile([P, 1], F32, tag=f"z{kk}", name="z_t")
                    nc.scalar.activation(
                        out=e_t[:, ts(kk, 1024)],
                        in_=s_ps,
                        func=AF.Exp,
                        scale=inv_exp,
                        accum_out=z_t,
                    )
                    zs.append(z_t)
                if gi + 1 < NH * QT_TILES:
                    pend_s = emit_S((gi + 1) // QT_TILES, (gi + 1) % QT_TILES)
                r_t = zp.tile([P, 1], F32, tag="r", name="r_t")
                nc.vector.tensor_add(r_t, zs[0], zs[1])
                nc.vector.reciprocal(r_t, r_t)
                rb16 = zp.tile([P, 4 * NH], BF, tag="rb", name="rb16")
                nc.vector.tensor_tensor(
                    rb16,
                    mask16,
                    r_t.to_broadcast((P, 4 * NH)),
                    mybir.AluOpType.mult,
                )
                bg_advance(gi)
                if pend_w is not None:
                    emit_w(pend_w)
                    if pend_w[4]:  # closed head pend_w[2]'s accumulator
                        aux.append(finalize(pend_w[2]))
                pend_w = (
                    e_t, rb16, gi // QT_TILES,
                    gi % QT_TILES == 0, gi % QT_TILES == QT_TILES - 1,
                )
                if gi == 52:
                    aux.append(attend(0))
                elif gi == 56:
                    aux.append(attend(1))
                elif gi == 60:
                    aux.append(attend(2))
                aux_step()

            emit_w(pend_w)
            aux.append(finalize(3))
            aux.append(attend(3))
            aux_step(n=100)
            nc.vector.tensor_copy(pooled2_sb, pooled_tile[0])
            nc.sync.dma_start(
                out=out_d.rearrange("a (b c) -> (a b) c", b=2),
                in_=pooled2_sb,
            )

    nc.finalize()
    return nc


def _get_nc():
    if "nc" not in _cache:
        _cache["nc"] = _build_nc()
    return _cache["nc"]


def _f8(a):
    return np.clip(a, -240.0, 240.0).astype(F8)


def _host_prep(inputs):
    """Build the 8 per-core input maps (shard + transpose + quantize)."""
    x = np.asarray(inputs["chunk_embeddings"], np.float32)
    wq = np.asarray(inputs["Wq"], np.float32)
    wk = np.asarray(inputs["Wk"], np.float32)
    wv = np.asarray(inputs["Wv"], np.float32)
    wo = np.asarray(inputs["Wo"], np.float32)
    bq = np.asarray(inputs["bq"], np.float32)
    in_maps = []
    for c in range(NCORES):
        b, hg = c // 2, c % 2
        sl = slice(hg * HGW, (hg + 1) * HGW)
        xT = x[b].T  # (1024, 2048): [po*128+pi, n]
        # xq8[pi, c, po, n'] = x[b, c*512+n', po*128+pi]
        xq8 = _f8(
            np.ascontiguousarray(
                xT.reshape(8, P, NCHUNK, 512).transpose(1, 2, 0, 3)
            )
        )
        # w?8[pi, h, po, d] = 32*W[hg*512+h*128+d, po*128+pi]
        def w8(W):
            m = (32.0 * W[sl, :]).T.reshape(8, P, NH, P).transpose(1, 2, 0, 3)
            return _f8(np.ascontiguousarray(m))
        # xbf[pi, po, n]
        xbf = np.ascontiguousarray(xT.reshape(8, P, N).transpose(1, 0, 2)).astype(
            BF16
        )
        # wvb[pi, po, o] = Wv[hg*512+o, po*128+pi]
        wvb = np.ascontiguousarray(
            wv[sl, :].T.reshape(8, P, HGW).transpose(1, 0, 2)
        ).astype(BF16)
        # wob[pi, h, o] = Wo[o, hg*512+h*128+pi]
        wob = np.ascontiguousarray(
            wo[:, sl].T.reshape(NH, P, HID).transpose(1, 0, 2)
        ).astype(BF16)
        bq32 = np.ascontiguousarray((32.0 * bq[sl]).reshape(NH, P).T)
        in_maps.append(
            {
                "xq8": xq8,
                "wq8": w8(wq),
                "wk8": w8(wk),
                "xbf": xbf,
                "wvb": wvb,
                "wob": wob,
                "bq32": bq32,
            }
        )
    return in_maps


def _unshard(results, inputs):
    bo = np.asarray(inputs["bo"], np.float32)
    bv = np.asarray(inputs["bv"], np.float32)
    Wo = np.asarray(inputs["Wo"], np.float32)
    bv_wo = Wo @ bv  # exact fold of the V bias through the output projection
    out = np.zeros((B, HID), np.float32)
    for b in range(B):
        out[b] = (
            results[2 * b]["out_pooled"][0]
            + results[2 * b + 1]["out_pooled"][0]
            + bv_wo
            + bo
        )
    return out


def _reference_numpy(inputs):
    """Fallback for non-trivial attention masks (never hit for the spec'd
    all-ones mask): straight numpy port of the reference."""
    x = np.asarray(inputs["chunk_embeddings"], np.float32)
    mask = np.asarray(inputs["attention_mask"])
    b, n, hid = x.shape

    def proj(W, bias):
        y = x @ np.asarray(W, np.float32).T + np.asarray(bias, np.float32)
        return y.reshape(b, n, HEADS, HD).transpose(0, 2, 1, 3)

    Q = proj(inputs["Wq"], inputs["bq"])
    K = proj(inputs["Wk"], inputs["bk"])
    V = proj(inputs["Wv"], inputs["bv"])
    s = np.einsum("bhqd,bhkd->bhqk", Q, K) / np.float32(np.sqrt(HD))
    s = np.where(mask[:, None, None, :] == 0, np.float32(-1e9), s)
    s = s - s.max(axis=-1, keepdims=True)
    e = np.exp(s)
    a = e / e.sum(axis=-1, keepdims=True)
    att = np.einsum("bhqk,bhkd->bhqd", a, V)
    att = att.transpose(0, 2, 1, 3).reshape(b, n, hid)
    out = att @ np.asarray(inputs["Wo"], np.float32).T + np.asarray(
        inputs["bo"], np.float32
    )
    m = mask[:, :, None].astype(np.float32)
    return (out * m).sum(axis=1) / m.sum(axis=1)


def _run(inputs, trace=False):
    from concourse.bass_utils import run_bass_kernel_spmd

    nc = _get_nc()
    in_maps = _host_prep(inputs)
    res = run_bass_kernel_spmd(
        nc, in_maps, core_ids=list(range(NCORES)), trace=trace
    )
    _cache["last_result"] = res
    return _unshard(res.results, inputs)


def kernel(**inputs):
    mask = np.asarray(inputs["attention_mask"])
    if not np.all(mask == 1):
        return _reference_numpy(inputs)
    return _run(inputs, trace=False)


def kernel_traced(**inputs):
    """Like kernel() but with NTFF profiling; returns (out, exec_time_ns)."""
    out = _run(inputs, trace=True)
    return out, _cache["last_result"].exec_time_ns


# revision 53
# speedup vs baseline: 1.1962x; 1.0177x over previous
"""AttentionPooling Trainium2 kernel (8 NeuronCores, Bass/Tile).

Sharding: (batch, head-group) — core c handles batch b=c//2 and heads
4*(c%2)..4*(c%2)+3. Each core computes, for its 4 heads, Q^T/K^T (head-dim
major) projections and V (token major), then a one-pass pooled attention:

  For each query stripe of 128 rows:  S = Q_stripe K^T  (PE, bf16)
  E = exp(S/(1024*sqrt(d))) (ScalarE, accum_out -> Z), r = 1/Z (VectorE)
  wacc[j, c] += onehot_j(r)^T E[:, j*512+c]  (PE, per-head [4,512] PSUM
  accumulator; the [4,128] block m transposes to w columns of k-tile 4j+m)

  attended_h = sum_t wT[k-tile t]^T V[t]  (PE, one-hot [4,128] accumulator)
  pooled = concat_h(attended) @ Wo_slice^T / N   (folded mean-pool)

Numerics: Q/K projections run fp8(e4m3) x fp8 with weights pre-scaled by
32 on the host (the 1/1024 folds into the exp scale) — fp8 matmuls run at
bf16 speed, but halve the critical-path DMA bytes. V stays bf16 (fp8 Wv
error does NOT average out through w@V); host-verified max rel err 5.6e-3
vs the fp32 reference. The K bias is dropped: it only adds a per-query
constant to the scores, which softmax cancels. V/output biases fold on the
host: pooled += Wo@bv + bo.

Schedule: a dummy exp preloads the ACT table and a few junk matmuls warm
the PE clock-gate while the critical DMAs land (fp8 x + h0 weights; the
bf16 x / Wv / Wo loads are dependency-deferred behind the prologue so they
don't steal DMA bandwidth). K(h0)+Q(h0,c0) project as a prologue; the
remaining Q/K chunks and V tiles interleave between attention stripes via
a cost/deadline-paced background queue. Per-head w finalizes (transpose to
wT) while the next head's stripes run; attended matmuls are slotted at
stripes 49/53/57; only head 3's finalize+attend and the 8 pooled matmuls
trail the last stripe.

PSUM (8 banks): S stripes 2x[128,1024] (4) + projection chunks 2x[128,512]
(2, also lent to small finalize tiles) + w/attended accumulators 2x (2).
"""

import math
import sys

import numpy as np

for _p in ("/opt/trn_rl_repo",):
    if _p not in sys.path:
        sys.path.append(_p)

import ml_dtypes

B, N, HID = 4, 2048, 1024
HEADS, HD = 8, 128
NH = 4          # heads per core
HGW = NH * HD   # head-group width (512)
NCORES = 8
P = 128
QT_TILES = N // P    # 16 query stripes per head
TOK_TILES = N // P   # 16 token tiles
NCHUNK = 4           # 512-token projection chunks

BF16 = ml_dtypes.bfloat16
F8 = ml_dtypes.float8_e4m3  # TRN fp8e4: max 240

_cache = {}


def _build_nc():
    import concourse.bacc as bacc
    import concourse.tile as tile
    from concourse import mybir
    from concourse.bass import ds, ts
    from concourse.masks import make_identity
    from concourse.tile import add_dep_helper

    BF = mybir.dt.bfloat16
    F32 = mybir.dt.float32
    FP8 = mybir.dt.float8e4
    AF = mybir.ActivationFunctionType

    nc = bacc.Bacc(trn_type="TRN2")

    # fp8 x, token-chunk major: xq8[pi, c, po, n'] = x[b, c*512+n', po*128+pi]
    xq8_d = nc.dram_tensor("xq8", (P, NCHUNK, 8, 512), FP8, kind="ExternalInput").ap()
    # fp8 Q/K weights (x32): wq8[pi, h, po, d] = 32*Wq[hg*512+h*128+d, po*128+pi]
    wq8_d = nc.dram_tensor("wq8", (P, NH, 8, P), FP8, kind="ExternalInput").ap()
    wk8_d = nc.dram_tensor("wk8", (P, NH, 8, P), FP8, kind="ExternalInput").ap()
    # bf16 x, hid-tile major: xbf[pi, po, n] = x[b, n, po*128+pi]
    xbf_d = nc.dram_tensor("xbf", (P, 8, N), BF, kind="ExternalInput").ap()
    # bf16 V weights: wvb[pi, po, o] = Wv[hg*512+o, po*128+pi]
    wvb_d = nc.dram_tensor("wvb", (P, 8, HGW), BF, kind="ExternalInput").ap()
    # bf16 O weights: wob[pi, h, o] = Wo[o, hg*512+h*128+pi]
    wob_d = nc.dram_tensor("wob", (P, NH, HID), BF, kind="ExternalInput").ap()
    bq_d = nc.dram_tensor("bq32", (P, NH), F32, kind="ExternalInput").ap()
    out_d = nc.dram_tensor("out_pooled", (1, HID), F32, kind="ExternalOutput").ap()

    inv_exp = float(1.0 / (1024.0 * math.sqrt(HD)))
    inv_pool = float(1.0 / N)

    with tile.TileContext(nc) as tc:
        with (
            tc.tile_pool(name="persist", bufs=1) as persist,
            tc.tile_pool(name="sp", bufs=2, space="PSUM") as sp,
            tc.tile_pool(name="pp", bufs=2, space="PSUM") as pp,
            tc.tile_pool(name="wp", bufs=2, space="PSUM") as wp,
            tc.tile_pool(name="ep", bufs=3) as ep,
            tc.tile_pool(name="zp", bufs=4) as zp,
        ):
            # ---- input DMAs, emitted first so the queues start at t0 ----
            # Per-queue FIFO tiering (no dep-gating — dep-gated DMAs degrade
            # to descriptor-at-a-time trickle): tier 1 = K(h0,c0) operands
            # across all 16 queues, tier 2 = remaining token chunks, tier 3
            # = everything the background projections need later.
            xq8_sb = [
                persist.tile([P, 8, 512], FP8, name=f"xq8_{i}")
                for i in range(NCHUNK)
            ]
            wq8_sb = [
                persist.tile([P, 8, P], FP8, name=f"wq8_{i}") for i in range(NH)
            ]
            wk8_sb = [
                persist.tile([P, 8, P], FP8, name=f"wk8_{i}") for i in range(NH)
            ]
            xbf_sb = persist.tile([P, 8, N], BF)
            wvb_sb = persist.tile([P, 8, HGW], BF)
            wob_sb = persist.tile([P, NH, HID], BF)
            bq_sb = persist.tile([P, NH], F32)

            def dma_split(dst, src_, nsplit):
                step = P // nsplit
                for i in range(nsplit):
                    nc.sync.dma_start(
                        out=dst[i * step : (i + 1) * step],
                        in_=src_[i * step : (i + 1) * step],
                    )

            # Each dma_start costs ~0.6us of serial issue on the Sync queue
            # (DIRECT2D), so the count is minimized and ordered so K(h0,c0)'s
            # operands issue first; transfers overlap later issues.
            nc.sync.dma_start(out=wk8_sb[0], in_=wk8_d[:, 0])
            dma_split(xq8_sb[0], xq8_d[:, 0], 2)
            nc.sync.dma_start(out=wq8_sb[0], in_=wq8_d[:, 0])
            for c in range(1, NCHUNK):
                nc.sync.dma_start(out=xq8_sb[c], in_=xq8_d[:, c])
            nc.sync.dma_start(out=bq_sb, in_=bq_d)
            for h in range(1, NH):
                nc.sync.dma_start(out=wk8_sb[h], in_=wk8_d[:, h])
                nc.sync.dma_start(out=wq8_sb[h], in_=wq8_d[:, h])
            for half in range(2):
                nc.sync.dma_start(
                    out=xbf_sb[:, 4 * half : 4 * half + 4, :],
                    in_=xbf_d[:, 4 * half : 4 * half + 4, :],
                )
            nc.sync.dma_start(out=wvb_sb, in_=wvb_d)
            nc.sync.dma_start(out=wob_sb, in_=wob_d)

            # ---- small constants (DVE) --------------------------------
            # mask16 columns {0,5,10,15} are 1: slicing [:, 4j:4j+4] gives
            # the one-hot column j used to route r into wacc row j.
            mask16 = persist.tile([P, 4 * NH], BF)
            nc.vector.memset(mask16, 0.0)
            for j in range(4):
                nc.vector.memset(mask16[:, 5 * j : 5 * j + 1], 1.0)
            zs128 = persist.tile([P, P], BF)
            nc.vector.memset(zs128, 0.0)
            ident4 = persist.tile([4, 4], F32)
            make_identity(nc, ident4)
            # 4x4 identity replicated at each 32-partition row group, so the
            # block transposes of wacc (stationary at base partition 32s) use
            # a moving operand at the same base partition.
            ident4x = persist.tile([P, 4], F32)
            nc.vector.memset(ident4x, 0.0)
            for s in range(4):
                nc.sync.dma_start(out=ident4x[32 * s : 32 * s + 4, :], in_=ident4)

            QT_sb = persist.tile([P, NH, N], BF)
            KT_sb = persist.tile([P, NH, N], BF)
            V_sb = persist.tile([P, TOK_TILES, HGW], BF)
            wacc_sb = persist.tile([P, 512], F32)
            # wT[pi, h, j, s, :]: [128,4] stationary for k-tile t=4j+s of
            # head h — one-hot at column j by construction (the transpose of
            # the block-diagonal wacc region), so head h's attended matmuls
            # accumulate partials into row j of a [4,128] PSUM tile.
            wT_sb = persist.tile([P, NH, 4, 4, 4], BF)
            att4_sb = persist.tile([4, P], F32)
            # attT2[:, h, oc, :]: [128,2] stationary with attended_h at
            # column oc (other column zero) so the two pooled-projection
            # matmuls of head h land in rows 0/1 of one [2,512] accumulator.
            attT2_sb = persist.tile([P, NH, 2, 2], BF)
            nc.vector.memset(attT2_sb, 0.0)
            pooled2_sb = persist.tile([2, 512], F32)

            # ---- ACT table preload + PE warmup (run under the DMAs) ---
            zdum = zp.tile([P, 16], BF, tag="zd", name="zdum")
            nc.scalar.activation(out=zdum, in_=mask16, func=AF.Exp)
            for _ in range(16):
                warm_ps = pp.tile([16, 256], F32, tag="proj", name="warm_ps")
                nc.tensor.matmul(
                    warm_ps, lhsT=mask16, rhs=KT_sb[:, 0, 0:256],
                    start=True, stop=True, skip_group_check=True,
                )

            # ---- projection emitters ----------------------------------
            def qk_chunk(proj_i, h, c, step=False):
                """512-token fp8 DoubleRow Q^T/K^T projection for head h:
                vitile v contracts hid pair-blocks (2v, 2v+1)."""
                wsb, dst = ((wq8_sb, QT_sb), (wk8_sb, KT_sb))[proj_i]
                ps = pp.tile([P, 512], F32, tag="proj", name="ps_qk")
                for v in range(4):
                    nc.tensor.matmul(
                        ps,
                        lhsT=wsb[h][:, 2 * v : 2 * v + 2, :],
                        rhs=xq8_sb[c][:, 2 * v : 2 * v + 2, :],
                        start=(v == 0),
                        stop=(v == 3),
                        perf_mode=mybir.MatmulPerfMode.DoubleRow,
                    )
                    if step and v == 1:
                        yield
                if proj_i == 0:
                    # Q bias (32*bq) folded into the psum->bf16 evacuation
                    ev = nc.vector.tensor_tensor(
                        dst[:, h, ts(c, 512)],
                        ps,
                        bq_sb[:, h : h + 1].to_broadcast((P, 512)),
                        mybir.AluOpType.add,
                    )
                else:
                    ev = nc.vector.tensor_copy(dst[:, h, ts(c, 512)], ps)
                if step:
                    yield ev

            def v_chunk(t, step=False):
                """128-token bf16 V projection tile (all 4 heads)."""
                ps = pp.tile([P, HGW], F32, tag="proj", name="ps_v")
                for i in range(8):
                    nc.tensor.matmul(
                        ps,
                        lhsT=xbf_sb[:, i, ts(t, P)],
                        rhs=wvb_sb[:, i, :],
                        start=(i == 0),
                        stop=(i == 7),
                    )
                    if step and i in (2, 5):
                        yield
                nc.vector.tensor_copy(V_sb[:, t, :], ps)
                if step:
                    yield

            # ---- prologue: K(h0) + Q(h0,c0) ---------------------------
            for c in range(NCHUNK):
                for _ in qk_chunk(1, 0, c):
                    pass
            for _ in qk_chunk(0, 0, 0):
                pass

            # ---- background queue: (generator, est_ns, deadline, nb) --
            bg = []
            for c in range(1, NCHUNK):
                bg.append((qk_chunk(0, 0, c, True), 1100.0, 4 * c - 2, 0))
            for h in range(1, NH):
                for c in range(NCHUNK):
                    bg.append((qk_chunk(1, h, c, True), 1100.0, 16 * h - 4 + c, 0))
                for c in range(NCHUNK):
                    bg.append(
                        (qk_chunk(0, h, c, True), 1100.0, 16 * h + 4 * c - 2, 0)
                    )
            for t in range(TOK_TILES):
                bg.append((v_chunk(t, True), 3000.0, 33 + t, 14 + t))
            bg_total = sum(u[1] for u in bg)
            bg_state = {"i": 0, "spent": 0.0}
            BG_SPREAD = 52  # finish all background work by stripe 52 of 64

            def bg_step():
                gen, cost, _, _ = bg[bg_state["i"]]
                try:
                    next(gen)
                    bg_state["spent"] += cost / 3.0
                except StopIteration:
                    bg_state["spent"] = sum(u[1] for u in bg[: bg_state["i"] + 1])
                    bg_state["i"] += 1

            def bg_advance(si):
                while bg_state["i"] < len(bg) and bg[bg_state["i"]][2] <= si + 1:
                    bg_step()
                target = (si + 1) * bg_total / BG_SPREAD
                while (
                    bg_state["i"] < len(bg)
                    and bg_state["spent"] < target
                    and bg[bg_state["i"]][3] <= si
                ):
                    bg_step()

            # ---- per-head finalize + attended (aux-paced) -------------
            wacc_tiles = {}
            pooled_tile = [None]

            def finalize(h):
                """wacc (PSUM, block-diag) -> wT_sb[:, h] one-hot k-tiles.
                The scale-copy (first step) releases the wacc pool slot; the
                16 transposes spread over the following stripes."""
                wps = wacc_tiles.pop(h)
                nc.vector.tensor_scalar_mul(wacc_sb, wps, inv_pool)
                yield
                for s in range(4):
                    for j in range(4):
                        tp = pp.tile([P, 4], F32, tag="proj", name="tp_ps")
                        nc.tensor.transpose(
                            tp,
                            wacc_sb[32 * s : 32 * s + 4, ts(j, P)],
                            ident4x[32 * s : 32 * s + 4, :],
                            tile_position=(32 * s, 0),
                        )
                        nc.vector.tensor_copy(wT_sb[:, h, j, s, :], tp)
                    yield

            def attend(h):
                """attended_h = sum_t wT[k-tile t]^T V[t, head h], then its
                two pooled-projection matmuls into the shared accumulator."""
                aps = pp.tile([4, P], F32, tag="proj", name="att4_ps")
                for t in range(TOK_TILES):
                    nc.tensor.matmul(
                        aps,
                        lhsT=wT_sb[:, h, t // 4, t % 4, :],
                        rhs=V_sb[:, t, ts(h, HD)],
                        start=(t == 0),
                        stop=(t == TOK_TILES - 1),
                    )
                    if t in (3, 7, 11):
                        yield
                nc.vector.tensor_copy(att4_sb, aps)
                atp = pp.tile([P, 4], F32, tag="proj", name="attT_ps")
                nc.tensor.transpose(atp, att4_sb, ident4)
                ar = zp.tile([P, 1], F32, tag="ar", name="attr")
                nc.vector.reduce_sum(ar, atp, axis=mybir.AxisListType.X)
                for oc in range(2):
                    nc.vector.tensor_copy(attT2_sb[:, h, oc, oc : oc + 1], ar)
                if pooled_tile[0] is None:
                    pooled_tile[0] = wp.tile([2, 512], F32, tag="w", name="pooled")
                for oc in range(2):
                    nc.tensor.matmul(
                        pooled_tile[0],
                        lhsT=attT2_sb[:, h, oc, :],
                        rhs=wob_sb[:, h, ts(oc, 512)],
                        start=(h == 0 and oc == 0),
                        stop=(h == NH - 1 and oc == 1),
                        skip_group_check=True,
                    )
                yield

            aux = []

            def aux_step(n=1):
                for _ in range(n):
                    while aux:
                        try:
                            next(aux[0])
                            break
                        except StopIteration:
                            aux.pop(0)

            # ---- pooled attention stripe loop -------------------------
            def emit_S(h, qi):
                tiles = []
                for kk in range(2):
                    s_ps = sp.tile([P, 1024], F32, tag="s", name="s_ps")
                    for kc in range(2):
                        nc.tensor.matmul(
                            s_ps[:, ts(kc, 512)],
                            lhsT=QT_sb[:, h, ts(qi, P)],
                            rhs=KT_sb[:, h, ds(kk * 1024 + kc * 512, 512)],
                            start=True,
                            stop=True,
                        )
                    tiles.append(s_ps)
                return tiles

            def emit_w(pend):
                # 16 [4,128] matmuls, 4-way col-group concurrent: region
                # (j, s) at partitions [32s, 32s+4), free [128j, 128j+128)
                # holds w[j*512+s*128+c] at row j (one-hot lhsT), i.e. the
                # [4,128] block (s, j) transposes to k-tile 4j+s.
                e_t, rb16, h, first, last = pend
                if first:
                    wacc_tiles[h] = wp.tile([P, 512], F32, tag="w", name="wacc")
                    # single full-bank zero-matmul opens the accumulation:
                    # start=True clearing is coarser than a [4,128] region,
                    # so per-region start bits would wipe sibling regions.
                    nc.tensor.matmul(
                        wacc_tiles[h],
                        lhsT=zs128,
                        rhs=e_t[:, 0:512],
                        start=True,
                        stop=False,
                        skip_group_check=True,
                    )
                wps = wacc_tiles[h]
                for j in range(4):
                    for s in range(4):
                        nc.tensor.matmul(
                            wps[32 * s : 32 * s + 4, ts(j, P)],
                            lhsT=rb16[:, 4 * j : 4 * j + 4],
                            rhs=e_t[:, ds(512 * j + 128 * s, P)],
                            start=False,
                            stop=last,
                            tile_position=(0, 32 * s),
                            skip_group_check=True,
                        )

            pend_s = emit_S(0, 0)
            pend_w = None
            for gi in range(NH * QT_TILES):
                e_t = ep.tile([P, N], BF, tag="e", name="e_t")
                zs = []
                for kk, s_ps in enumerate(pend_s):
                    z_t = zp.tile([P, 1], F32, tag=f"z{kk}", name="z_t")
                    nc.scalar.activation(
                        out=e_t[:, ts(kk, 1024)],
                        in_=s_ps,
                        func=AF.Exp,
                        scale=inv_exp,
                        accum_out=z_t,
                    )
                    zs.append(z_t)
                if gi + 1 < NH * QT_TILES:
                    pend_s = emit_S((gi + 1) // QT_TILES, (gi + 1) % QT_TILES)
                r_t = zp.tile([P, 1], F32, tag="r", name="r_t")
                nc.vector.tensor_add(r_t, zs[0], zs[1])
                nc.vector.reciprocal(r_t, r_t)
                rb16 = zp.tile([P, 4 * NH], BF, tag="rb", name="rb16")
                nc.vector.tensor_tensor(
                    rb16,
                    mask16,
                    r_t.to_broadcast((P, 4 * NH)),
                    mybir.AluOpType.mult,
                )
                bg_advance(gi)
                if pend_w is not None:
                    emit_w(pend_w)
                    if pend_w[4]:  # closed head pend_w[2]'s accumulator
                        aux.append(finalize(pend_w[2]))
                pend_w = (
                    e_t, rb16, gi // QT_TILES,
                    gi % QT_TILES == 0, gi % QT_TILES == QT_TILES - 1,
                )
                if gi == 52:
                    aux.append(attend(0))
                elif gi == 56:
                    aux.append(attend(1))
                elif gi == 60:
                    aux.append(attend(2))
                aux_step()

            emit_w(pend_w)
            aux.append(finalize(3))
            aux.append(attend(3))
            aux_step(n=100)
            nc.vector.tensor_copy(pooled2_sb, pooled_tile[0])
            nc.sync.dma_start(
                out=out_d.rearrange("a (b c) -> (a b) c", b=2),
                in_=pooled2_sb,
            )

    nc.finalize()
    return nc


def _get_nc():
    if "nc" not in _cache:
        _cache["nc"] = _build_nc()
    return _cache["nc"]


def _f8(a):
    return np.clip(a, -240.0, 240.0).astype(F8)


def _host_prep(inputs):
    """Build the 8 per-core input maps (shard + transpose + quantize)."""
    x = np.asarray(inputs["chunk_embeddings"], np.float32)
    wq = np.asarray(inputs["Wq"], np.float32)
    wk = np.asarray(inputs["Wk"], np.float32)
    wv = np.asarray(inputs["Wv"], np.float32)
    wo = np.asarray(inputs["Wo"], np.float32)
    bq = np.asarray(inputs["bq"], np.float32)
    in_maps = []
    for c in range(NCORES):
        b, hg = c // 2, c % 2
        sl = slice(hg * HGW, (hg + 1) * HGW)
        xT = x[b].T  # (1024, 2048): [po*128+pi, n]
        # xq8[pi, c, po, n'] = x[b, c*512+n', po*128+pi]
        xq8 = _f8(
            np.ascontiguousarray(
                xT.reshape(8, P, NCHUNK, 512).transpose(1, 2, 0, 3)
            )
        )
        # w?8[pi, h, po, d] = 32*W[hg*512+h*128+d, po*128+pi]
        def w8(W):
            m = (32.0 * W[sl, :]).T.reshape(8, P, NH, P).transpose(1, 2, 0, 3)
            return _f8(np.ascontiguousarray(m))
        # xbf[pi, po, n]
        xbf = np.ascontiguousarray(xT.reshape(8, P, N).transpose(1, 0, 2)).astype(
            BF16
        )
        # wvb[pi, po, o] = Wv[hg*512+o, po*128+pi]
        wvb = np.ascontiguousarray(
            wv[sl, :].T.reshape(8, P, HGW).transpose(1, 0, 2)
        ).astype(BF16)
        # wob[pi, h, o] = Wo[o, hg*512+h*128+pi]
        wob = np.ascontiguousarray(
            wo[:, sl].T.reshape(NH, P, HID).transpose(1, 0, 2)
        ).astype(BF16)
        bq32 = np.ascontiguousarray((32.0 * bq[sl]).reshape(NH, P).T)
        in_maps.append(
            {
                "xq8": xq8,
                "wq8": w8(wq),
                "wk8": w8(wk),
                "xbf": xbf,
                "wvb": wvb,
                "wob": wob,
                "bq32": bq32,
            }
        )
    return in_maps


def _unshard(results, inputs):
    bo = np.asarray(inputs["bo"], np.float32)
    bv = np.asarray(inputs["bv"], np.float32)
    Wo = np.asarray(inputs["Wo"], np.float32)
    bv_wo = Wo @ bv  # exact fold of the V bias through the output projection
    out = np.zeros((B, HID), np.float32)
    for b in range(B):
        out[b] = (
            results[2 * b]["out_pooled"][0]
            + results[2 * b + 1]["out_pooled"][0]
            + bv_wo
            + bo
        )
    return out


def _reference_numpy(inputs):
    """Fallback for non-trivial attention masks (never hit for the spec'd
    all-ones mask): straight numpy port of the reference."""
    x = np.asarray(inputs["chunk_embeddings"], np.float32)
    mask = np.asarray(inputs["attention_mask"])
    b, n, hid = x.shape

    def proj(W, bias):
        y = x @ np.asarray(W, np.float32).T + np.asarray(bias, np.float32)
        return y.reshape(b, n, HEADS, HD).transpose(0, 2, 1, 3)

    Q = proj(inputs["Wq"], inputs["bq"])
    K = proj(inputs["Wk"], inputs["bk"])
    V = proj(inputs["Wv"], inputs["bv"])
    s = np.einsum("bhqd,bhkd->bhqk", Q, K) / np.float32(np.sqrt(HD))
    s = np.where(mask[:, None, None, :] == 0, np.float32(-1e9), s)
    s = s - s.max(axis=-1, keepdims=True)
    e = np.exp(s)
    a = e / e.sum(axis=-1, keepdims=True)
    att = np.einsum("bhqk,bhkd->bhqd", a, V)
    att = att.transpose(0, 2, 1, 3).reshape(b, n, hid)
    out = att @ np.asarray(inputs["Wo"], np.float32).T + np.asarray(
        inputs["bo"], np.float32
    )
    m = mask[:, :, None].astype(np.float32)
    return (out * m).sum(axis=1) / m.sum(axis=1)


def _run(inputs, trace=False):
    from concourse.bass_utils import run_bass_kernel_spmd

    nc = _get_nc()
    in_maps = _host_prep(inputs)
    res = run_bass_kernel_spmd(
        nc, in_maps, core_ids=list(range(NCORES)), trace=trace
    )
    _cache["last_result"] = res
    return _unshard(res.results, inputs)


def kernel(**inputs):
    mask = np.asarray(inputs["attention_mask"])
    if not np.all(mask == 1):
        return _reference_numpy(inputs)
    return _run(inputs, trace=False)


def kernel_traced(**inputs):
    """Like kernel() but with NTFF profiling; returns (out, exec_time_ns)."""
    out = _run(inputs, trace=True)
    return out, _cache["last_result"].exec_time_ns
